# revision 1
# baseline (speedup 1.0000x reference)
"""GAT layer on 8 Trainium2 NeuronCores (Bass/Tile).

Strategy (target-per-partition layout, edge gathers via ANT dma_gather):
  - Targets sharded across 8 cores (12500 each). Per core, targets are
    reordered (lex4 clustering by per-chunk source counts) into 98 blocks
    of 128; block b target v lives on SBUF partition v.
  - Phase B builds a DRAM table row per node: [p bf16(128) | alpha_src
    f32(8) | junk] = 512B rows, via PE matmuls (stationary = xT tile,
    moving = [W_proj | W_proj @ Ablk]).
  - Per block, per source-chunk (4 chunks of <=25088 nodes so indices fit
    int16), dma_gather pulls each edge slot's row into [128 targets, W,
    256] bf16 tiles. Padding slots point at a per-chunk sentinel row
    (p=0, alpha=-300 -> exp(s)~0).
  - s = alpha + beta[target] (beta broadcast per partition), Lrelu(0.2),
    Exp -> E. Factored softmax: U = sum_d E*p, D = sum_d E, out =
    U/D + skip, then ELU. beta/skip come from a per-block matmul with
    stationary = xTperm tile, moving = [W_skip | W_proj @ Bblk].
"""

import os
import sys

sys.path.insert(0, "/opt/trn_rl_repo")

import numpy as np
from contextlib import ExitStack

import concourse.bass as bass
import concourse.bacc as bacc
import concourse.tile as tile
from concourse import mybir
from concourse._compat import cdiv
from concourse.bass_utils import run_bass_kernel_spmd
from concourse.library_config import mlp

N_NODES = 100000
N_EDGES = 1600000
IN_F = 128
H = 8
F = 16
HF = H * F  # 128
NEG_SLOPE = 0.2
EPS = 1e-16
N_CORES = 8
TGT_PER_CORE = N_NODES // N_CORES  # 12500
N_BLOCKS = cdiv(TGT_PER_CORE, 128)  # 98
TGT_PAD = N_BLOCKS * 128  # 12544
CHUNK = 196 * 128  # 25088 nodes per chunk (tile aligned)
N_CHUNKS = 4
CHUNK_NODES = [CHUNK, CHUNK, CHUNK, N_NODES - 3 * CHUNK]  # last: 24736
CHUNK_ROWS = [n + 1 for n in CHUNK_NODES]  # +1 sentinel row per chunk
CHUNK_BASE = [0]
for _c in range(1, N_CHUNKS):
    CHUNK_BASE.append(CHUNK_BASE[-1] + CHUNK_ROWS[_c - 1])
TABLE_ROWS = CHUNK_BASE[-1] + CHUNK_ROWS[-1]  # 100004
ROW_ELEMS = 256  # bf16 elems per table row (512B)
ALPHA_SENT = -300.0
N_TILES = N_NODES // 128  # 781.25 -> handled per chunk: 196,196,196,193.25?
TILES_PER_CHUNK = [n // 128 for n in CHUNK_NODES]  # [196,196,196,193]
LAST_PARTIAL = CHUNK_NODES[3] - TILES_PER_CHUNK[3] * 128  # 24736-24704=32

_COMPILED = {}


def _host_prep(x, edge_index, W_proj, W_skip, a_src, a_tgt):
    """Pure index/layout prep. Returns (common, per_core list)."""
    x = np.asarray(x, np.float32)
    ei = np.asarray(edge_index)
    src = ei[0].astype(np.int64)
    tgt = ei[1].astype(np.int64)

    xT = np.ascontiguousarray(x.T)  # [128, N]
    WprojT = np.ascontiguousarray(np.asarray(W_proj, np.float32).T)
    # block-diagonal score layouts: Ablk[hf, h] = a_src[h, f'] at hf=h*16+f'
    Ablk = np.zeros((HF, H), np.float32)
    Bblk = np.zeros((HF, H), np.float32)
    asr = np.asarray(a_src, np.float32).reshape(H, F)
    atg = np.asarray(a_tgt, np.float32).reshape(H, F)
    for h in range(H):
        Ablk[h * F:(h + 1) * F, h] = asr[h]
        Bblk[h * F:(h + 1) * F, h] = atg[h]
    ABblk = np.concatenate([Ablk, Bblk], axis=1)  # [128, 16]

    chunk_of = np.minimum(src // CHUNK, N_CHUNKS - 1)
    local_of = (src - chunk_of * CHUNK).astype(np.int64)

    cores = []
    for c in range(N_CORES):
        lo, hi = c * TGT_PER_CORE, (c + 1) * TGT_PER_CORE
        m = (tgt >= lo) & (tgt < hi)
        s_loc = local_of[m]
        s_ch = chunk_of[m]
        t_loc = (tgt[m] - lo).astype(np.int64)
        cnt = np.zeros((TGT_PER_CORE, N_CHUNKS), np.int32)
        np.add.at(cnt, (t_loc, s_ch), 1)
        order = np.lexsort((-cnt[:, 3], -cnt[:, 2], -cnt[:, 1], -cnt[:, 0]))
        rank = np.empty(TGT_PER_CORE, np.int64)
        rank[order] = np.arange(TGT_PER_CORE)
        cores.append(dict(order=order, rank=rank, cnt=cnt,
                          s_loc=s_loc, s_ch=s_ch, t_loc=t_loc))

    # common schedule: W[b, ch] = max over cores of per-block max count
    W = np.zeros((N_BLOCKS, N_CHUNKS), np.int32)
    for c in range(N_CORES):
        d = cores[c]
        cnt_ord = d["cnt"][d["order"]]  # [12500, 4]
        cnt_pad = np.zeros((TGT_PAD, N_CHUNKS), np.int32)
        cnt_pad[:TGT_PER_CORE] = cnt_ord
        blkmax = cnt_pad.reshape(N_BLOCKS, 128, N_CHUNKS).max(axis=1)
        W = np.maximum(W, blkmax)
    W[:, 0] = np.maximum(W[:, 0], 1)  # ensure every block has >=1 slot

    per_core = []
    for c in range(N_CORES):
        d = cores[c]
        # slot fill: for each (rank, ch) edges in arbitrary order
        rk = d["rank"][d["t_loc"]]  # rank of each edge's target
        # sort edges by (rank, ch) to place into slots
        eo = np.lexsort((d["s_ch"], rk))
        rk_s = rk[eo]
        ch_s = d["s_ch"][eo]
        sl_s = d["s_loc"][eo]
        # position of each edge within its (rank, ch) group
        key = rk_s * N_CHUNKS + ch_s
        uk = np.unique(key)
        firsts = np.searchsorted(key, uk)
        dpos = np.arange(len(key)) - firsts[np.searchsorted(uk, key)]
        # build idx arrays per (block, ch): [W, 128] (d-major, partition minor)
        idx_cols = []
        for b in range(N_BLOCKS):
            for ch in range(N_CHUNKS):
                w = int(W[b, ch])
                if w == 0:
                    continue
                arr = np.full((w, 128), CHUNK_NODES[ch], np.int64)  # sentinel
                mm = (rk_s // 128 == b) & (ch_s == ch)
                if mm.any():
                    p = (rk_s[mm] % 128).astype(np.int64)
                    dd = dpos[mm]
                    arr[dd, p] = sl_s[mm]
                flat = arr.reshape(-1)  # j = d*128 + p
                wrap = flat.reshape(-1, 16).T  # [16, 8w]
                idx_cols.append(np.tile(wrap, (8, 1)))  # [128, 8w]
        idxs = np.concatenate(idx_cols, axis=1).astype(np.int16)  # [128, C]

        perm = d["order"]
        xTp = np.zeros((IN_F, TGT_PAD), np.float32)
        xTp[:, :TGT_PER_CORE] = x[perm + c * TGT_PER_CORE].T
        per_core.append(dict(idxs=idxs, xTperm=np.ascontiguousarray(xTp),
                             perm=perm))

    common = dict(xT=xT, WprojT=WprojT, ABblk=ABblk,
                  Wproj=np.asarray(W_proj, np.float32),
                  Wskip=np.asarray(W_skip, np.float32), W=W)
    return common, per_core


def _build_program(W):
    nc = bacc.Bacc("TRN2", debug=False, num_devices=N_CORES,
                   num_swdge_queues=4)
    f32 = mybir.dt.float32
    bf16 = mybir.dt.bfloat16
    i16 = mybir.dt.int16

    C_total = int(8 * W.sum())
    xT = nc.dram_tensor("xT", [IN_F, N_NODES], f32, kind="ExternalInput").ap()
    xTperm = nc.dram_tensor("xTperm", [IN_F, TGT_PAD], f32,
                            kind="ExternalInput").ap()
    WprojT_d = nc.dram_tensor("WprojT", [HF, IN_F], f32,
                              kind="ExternalInput").ap()
    ABblk_d = nc.dram_tensor("ABblk", [HF, 2 * H], f32,
                             kind="ExternalInput").ap()
    Wproj_d = nc.dram_tensor("Wproj", [IN_F, HF], f32,
                             kind="ExternalInput").ap()
    Wskip_d = nc.dram_tensor("Wskip", [IN_F, HF], f32,
                             kind="ExternalInput").ap()
    idxs_d = nc.dram_tensor("idxs", [128, C_total], i16,
                            kind="ExternalInput").ap()
    out_d = nc.dram_tensor("out", [TGT_PAD, HF], f32,
                           kind="ExternalOutput").ap()
    table = nc.dram_tensor("table", [TABLE_ROWS, ROW_ELEMS], bf16).ap()

    with tile.TileContext(nc) as tc, ExitStack() as ctx:
        consts = ctx.enter_context(tc.tile_pool(name="consts", bufs=1))
        sb = ctx.enter_context(tc.tile_pool(name="sb", bufs=3))
        stg = ctx.enter_context(tc.tile_pool(name="stg", bufs=4))
        gpool = ctx.enter_context(tc.tile_pool(name="gpool", bufs=3))
        work = ctx.enter_context(tc.tile_pool(name="work", bufs=2))
        psum = ctx.enter_context(tc.tile_pool(name="psum", bufs=2,
                                              space="PSUM"))
        psum2 = ctx.enter_context(tc.tile_pool(name="psum2", bufs=2,
                                               space="PSUM"))
        idxp = ctx.enter_context(tc.tile_pool(name="idxp", bufs=2))

        nc.gpsimd.load_library(mlp)

        # --- Phase A: weights and packs -------------------------------
        wprojT_t = consts.tile([HF, IN_F], f32)
        nc.sync.dma_start(out=wprojT_t[:], in_=WprojT_d[:])
        abblk_t = consts.tile([HF, 2 * H], f32)
        nc.sync.dma_start(out=abblk_t[:], in_=ABblk_d[:])
        pack0 = consts.tile([IN_F, HF + H], f32)  # [W_proj | Wa]
        pack2 = consts.tile([IN_F, HF + H], f32)  # [W_skip | Wb]
        nc.sync.dma_start(out=pack0[:, :HF], in_=Wproj_d[:])
        nc.sync.dma_start(out=pack2[:, :HF], in_=Wskip_d[:])
        wab_ps = psum.tile([IN_F, 2 * H], f32, space="PSUM")
        nc.tensor.matmul(out=wab_ps[:], lhsT=wprojT_t[:],
                         rhs=abblk_t[:], start=True, stop=True)
        nc.vector.tensor_copy(out=pack0[:, HF:HF + H], in_=wab_ps[:, 0:H])
        nc.vector.tensor_copy(out=pack2[:, HF:HF + H], in_=wab_ps[:, H:2 * H])

        # --- Phase B: build table ------------------------------------
        for ch in range(N_CHUNKS):
            ntile = cdiv(CHUNK_NODES[ch], 128)
            for t in range(ntile):
                nrows = min(128, CHUNK_NODES[ch] - t * 128)
                n0 = ch * CHUNK + t * 128
                xt = stg.tile([IN_F, 128], f32, tag="xt")
                nc.sync.dma_start(out=xt[:, :nrows],
                                  in_=xT[:, n0:n0 + nrows])
                pa = psum.tile([128, HF + H], f32, space="PSUM", tag="pa")
                nc.tensor.matmul(out=pa[:nrows], lhsT=xt[:, :nrows],
                                 rhs=pack0[:], start=True, stop=True)
                row = stg.tile([128, ROW_ELEMS], bf16, tag="row")
                nc.scalar.activation(out=row[:nrows, 0:HF],
                                     in_=pa[:nrows, 0:HF],
                                     func=mybir.ActivationFunctionType.Copy)
                nc.vector.tensor_copy(
                    out=row[:nrows, HF:HF + 2 * H].bitcast(f32),
                    in_=pa[:nrows, HF:HF + H])
                r0 = CHUNK_BASE[ch] + t * 128
                nc.sync.dma_start(out=table[r0:r0 + nrows, :],
                                  in_=row[:nrows, :])
            # sentinel row for this chunk
            srow = stg.tile([1, ROW_ELEMS], bf16, tag="sent")
            nc.gpsimd.memset(srow[:, 0:HF], 0.0)
            nc.gpsimd.memset(srow[:, HF:HF + 2 * H].bitcast(f32), ALPHA_SENT)
            nc.gpsimd.memset(srow[:, HF + 2 * H:].bitcast(f32), 0.0)
            sr = CHUNK_BASE[ch] + CHUNK_NODES[ch]
            nc.sync.dma_start(out=table[sr:sr + 1, :], in_=srow[:])

        # --- Phase C: per-block edge processing ----------------------
        Wsum = W.sum(axis=1)
        col_off = np.concatenate(([0], np.cumsum(8 * W.reshape(-1))))
        call_i = 0
        IDX_GRP = 8  # blocks per idx load group
        for b in range(N_BLOCKS):
            if b % IDX_GRP == 0:
                g0 = int(col_off[b * N_CHUNKS])
                g1 = int(col_off[min(b + IDX_GRP, N_BLOCKS) * N_CHUNKS])
                idx_t = idxp.tile([128, g1 - g0], i16, tag="idxg")
                nc.sync.dma_start(out=idx_t[:], in_=idxs_d[:, g0:g1])
                grp_base = g0
            sw = int(Wsum[b])
            # skip | beta matmul
            sk_ps = psum2.tile([128, HF + H], f32, space="PSUM", tag="sk")
            xp = stg.tile([IN_F, 128], f32, tag="xp")
            nc.sync.dma_start(out=xp[:], in_=xTperm[:, b * 128:(b + 1) * 128])
            nc.tensor.matmul(out=sk_ps[:], lhsT=xp[:], rhs=pack2[:],
                             start=True, stop=True)
            beta = work.tile([128, H], f32, tag="beta")
            nc.vector.tensor_copy(out=beta[:], in_=sk_ps[:, HF:HF + H])

            # gathers into G [128, sw, 256]
            G = gpool.tile([128, sw, ROW_ELEMS], bf16, tag="G")
            doff = 0
            for ch in range(N_CHUNKS):
                w = int(W[b, ch])
                if w == 0:
                    continue
                ni = 128 * w
                c0 = int(col_off[b * N_CHUNKS + ch]) - grp_base
                tab_ch = table[CHUNK_BASE[ch]:CHUNK_BASE[ch] + CHUNK_ROWS[ch], :]
                nc.gpsimd.dma_gather(
                    G[:, doff:doff + w, :],
                    tab_ch,
                    idx_t[:, c0:c0 + 8 * w],
                    ni, ni, ROW_ELEMS,
                    single_packet=False,
                    queue_num=call_i % 4,
                )
                call_i += 1
                doff += w
            # alpha view: [128, sw, 8] f32
            al = G[:, :, HF:HF + 2 * H].bitcast(f32)
            s_t = work.tile([128, sw, H], f32, tag="s")
            nc.vector.tensor_tensor(
                out=s_t[:], in0=al,
                in1=beta[:].unsqueeze(1).to_broadcast([128, sw, H]),
                op=mybir.AluOpType.add)
            # exp(lrelu(z)) = exp(0.2 z) * exp(0.8 relu(z)); ACT Lrelu has a
            # fixed 0.01 slope so build it from Relu/Exp(scale=...) instead.
            e1 = work.tile([128, sw, H], f32, tag="e1")
            nc.scalar.activation(out=e1[:], in_=s_t[:],
                                 func=mybir.ActivationFunctionType.Exp,
                                 scale=NEG_SLOPE)
            r_t = work.tile([128, sw, H], f32, tag="rt")
            nc.scalar.activation(out=r_t[:], in_=s_t[:],
                                 func=mybir.ActivationFunctionType.Relu)
            e2 = work.tile([128, sw, H], f32, tag="e2")
            nc.scalar.activation(out=e2[:], in_=r_t[:],
                                 func=mybir.ActivationFunctionType.Exp,
                                 scale=1.0 - NEG_SLOPE)
            Ebf = work.tile([128, sw, H], bf16, tag="Ebf")
            nc.vector.tensor_tensor(out=Ebf[:], in0=e1[:], in1=e2[:],
                                    op=mybir.AluOpType.mult)
            # expand E to [128, sw, 128]
            Ex = work.tile([128, sw, H, F], bf16, tag="Ex")
            nc.vector.tensor_copy(
                out=Ex[:],
                in_=Ebf[:].unsqueeze(3).to_broadcast([128, sw, H, F]))
            # M = p * Ex
            M = work.tile([128, sw, HF], bf16, tag="M")
            nc.vector.tensor_tensor(out=M[:], in0=G[:, :, 0:HF], in1=Ex[:].rearrange("p w h f -> p w (h f)"),
                                    op=mybir.AluOpType.mult)
            # U[v, f] = sum_d M ; D[v, h] = sum_d E
            U = work.tile([128, HF], f32, tag="U")
            nc.vector.tensor_reduce(out=U[:], in_=M[:].transpose([0, 2, 1]),
                                    axis=mybir.AxisListType.X,
                                    op=mybir.AluOpType.add)
            D = work.tile([128, H], f32, tag="D")
            nc.vector.tensor_reduce(out=D[:], in_=Ebf[:].transpose([0, 2, 1]),
                                    axis=mybir.AxisListType.X,
                                    op=mybir.AluOpType.add)
            Dinv = work.tile([128, H], f32, tag="Dinv")
            nc.vector.tensor_scalar_add(Dinv[:], D[:], EPS)
            nc.vector.reciprocal(out=Dinv[:], in_=Dinv[:])
            O = work.tile([128, HF], f32, tag="O")
            nc.vector.tensor_tensor(
                out=O[:].rearrange("p (h f) -> p h f", h=H),
                in0=U[:].rearrange("p (h f) -> p h f", h=H),
                in1=Dinv[:].unsqueeze(2).to_broadcast([128, H, F]),
                op=mybir.AluOpType.mult)
            nc.vector.tensor_tensor(out=O[:], in0=O[:], in1=sk_ps[:, 0:HF],
                                    op=mybir.AluOpType.add)
            # ELU = (max(O,0) - 1) + exp(min(O,0))
            A_t = work.tile([128, HF], f32, tag="At")
            nc.vector.tensor_scalar(out=A_t[:], in0=O[:], scalar1=0.0,
                                    scalar2=-1.0, op0=mybir.AluOpType.max,
                                    op1=mybir.AluOpType.add)
            T_t = work.tile([128, HF], f32, tag="Tt")
            nc.vector.tensor_scalar_min(T_t[:], O[:], 0.0)
            E2 = work.tile([128, HF], f32, tag="E2")
            nc.scalar.activation(out=E2[:], in_=T_t[:],
                                 func=mybir.ActivationFunctionType.Exp)
            OUT = work.tile([128, HF], f32, tag="OUT")
            nc.vector.tensor_tensor(out=OUT[:], in0=A_t[:], in1=E2[:],
                                    op=mybir.AluOpType.add)
            nc.sync.dma_start(out=out_d[b * 128:(b + 1) * 128, :],
                              in_=OUT[:])

    nc.compile()
    return nc


def kernel(x, edge_index, W_proj, W_skip, a_src, a_tgt):
    common, per_core = _host_prep(x, edge_index, W_proj, W_skip, a_src, a_tgt)
    key = "prog"
    if key not in _COMPILED:
        _COMPILED[key] = _build_program(common["W"])
    nc = _COMPILED[key]

    in_maps = []
    for c in range(N_CORES):
        pc = per_core[c]
        in_maps.append({
            "xT": common["xT"],
            "xTperm": pc["xTperm"],
            "WprojT": common["WprojT"],
            "ABblk": common["ABblk"],
            "Wproj": common["Wproj"],
            "Wskip": common["Wskip"],
            "idxs": pc["idxs"],
        })
    trace = bool(int(os.environ.get("GAT_TRACE", "0")))
    res = run_bass_kernel_spmd(nc, in_maps, list(range(N_CORES)),
                               trace=trace)
    if trace:
        kernel.last_exec_time_ns = res.exec_time_ns
        kernel.last_mean_exec_time_ns = res.mean_exec_time_ns

    out = np.empty((N_NODES, HF), np.float32)
    for c in range(N_CORES):
        o = res.results[c]["out"]  # [12544, 128] in rank order
        perm = per_core[c]["perm"]
        out[c * TGT_PER_CORE + perm] = o[:TGT_PER_CORE]
    return out


kernel.last_exec_time_ns = None
kernel.last_mean_exec_time_ns = None



# revision 3
# speedup vs baseline: 1.0487x; 1.0487x over previous
"""GAT layer on 8 Trainium2 NeuronCores (Bass/Tile) — v2.

Target-per-partition layout (like baseline) with:
  - global count-sorted target clustering dealt round-robin to cores
    (shared static W = max over 8 cores per slot),
  - per-core runtime gather counts (trailing -1 idxs skipped by the
    SWDGE ucode) => descriptor count ~1.35x E instead of 2.06x,
  - bf16 phase B with fat (multi-KB) DMA descriptors via a
    partition-major table layout,
  - mask-based padding (no sentinel rows),
  - DVE tree-reduce instead of transposed tensor_reduce,
  - 4D inner-broadcast tensor_tensor (no Ex materialization).
"""

import os
import sys

sys.path.insert(0, "/opt/trn_rl_repo")

import numpy as np
from contextlib import ExitStack

import concourse.bass as bass
import concourse.bacc as bacc
import concourse.tile as tile
from concourse import mybir
from concourse.bass_utils import run_bass_kernel_spmd
from concourse.library_config import mlp

N_NODES = 100000
N_EDGES = 1600000
IN_F = 128
H = 8
F = 16
HF = H * F
NEG_SLOPE = 0.2
EPS = 1e-16
N_CORES = 8
N_BLOCKS = 98
N_GBLK = 784  # global blocks of 128 sorted targets
NCH = 4
CH_NODES = [25088, 25088, 25088, 24736]
T_CH = [196, 196, 196, 194]
CH_BASE_N = [0, 25088, 50176, 75264]
CH_ROWS = [128 * t for t in T_CH]  # 25088,25088,25088,24832
CH_BASE_R = [0, 25088, 50176, 75264]
TABLE_ROWS = CH_BASE_R[-1] + CH_ROWS[-1]  # 100096
ROW_E = 256  # bf16 elems per table row (512B)
XT_COLS = 100352  # 784*128, padded
ALPHA_PAD = -300.0
G_BUFS = 3
SINGLE_PACKET = False

_COMPILED = {}


def _host_prep(x, edge_index, W_proj, W_skip, a_src, a_tgt):
    x = np.asarray(x, np.float32)
    ei = np.asarray(edge_index).astype(np.int64)
    src, tgt = ei[0], ei[1]

    ch_of = np.minimum(src // 25088, 3)
    l = src - np.take(CH_BASE_N, ch_of)
    row_of = (l % 128) * np.take(T_CH, ch_of) + l // 128  # local row in chunk

    cnt = np.zeros((N_NODES, NCH), np.int32)
    np.add.at(cnt, (tgt, ch_of), 1)
    order = np.lexsort((-cnt[:, 3], -cnt[:, 2], -cnt[:, 1], -cnt[:, 0]))
    rank = np.empty(N_NODES, np.int64)
    rank[order] = np.arange(N_NODES)
    blk = rank // 128  # global block of each target
    pos = rank % 128
    core_of = blk % 8
    slot_of = blk // 8

    cnt_sorted = np.zeros((N_GBLK * 128, NCH), np.int32)
    cnt_sorted[:N_NODES] = cnt[order]
    bm = cnt_sorted.reshape(N_GBLK, 128, NCH).max(axis=1)  # per global block
    W = bm.reshape(N_BLOCKS, 8, NCH).max(axis=1).astype(np.int32)
    Wsum = W.sum(axis=1)
    Wmax = int(Wsum.max())

    # weight packs (host-side repack of params only)
    Ablk = np.zeros((HF, H), np.float32)
    Bblk = np.zeros((HF, H), np.float32)
    asr = np.asarray(a_src, np.float32).reshape(H, F)
    atg = np.asarray(a_tgt, np.float32).reshape(H, F)
    for h in range(H):
        Ablk[h * F:(h + 1) * F, h] = asr[h]
        Bblk[h * F:(h + 1) * F, h] = atg[h]
    Wp = np.asarray(W_proj, np.float32)
    pack0 = np.concatenate([Wp, Wp @ Ablk], axis=1)  # [128,136]
    pack2 = np.concatenate([np.asarray(W_skip, np.float32), Wp @ Bblk], axis=1)

    import jax.numpy as jnp

    def to_bf16_bytes(a):
        return np.asarray(jnp.asarray(a, jnp.bfloat16)).view(np.uint16)

    xT = np.zeros((IN_F, XT_COLS), np.float32)
    xT[:, :N_NODES] = x.T
    xT_bf = to_bf16_bytes(xT[:, :TABLE_ROWS])  # device input [128, 100096] u16
    pack0_bf = to_bf16_bytes(pack0)
    pack2_bf = to_bf16_bytes(pack2)

    # per-(core, slot, chunk) edge placement
    e_core = core_of[tgt]
    e_slot = slot_of[tgt]
    e_pos = pos[tgt]
    e_ch = ch_of

    col_off = np.zeros(N_BLOCKS * NCH + 1, np.int64)  # idx col offsets (8*W)
    np.cumsum(8 * W.reshape(-1), out=col_off[1:])
    IDX_COLS = int(col_off[-1])
    MASK_COLS = int(Wsum.sum())

    per_core = []
    for c in range(N_CORES):
        m = e_core == c
        j_e = e_slot[m]
        p_e = e_pos[m]
        ch_e = e_ch[m]
        r_e = row_of[m]
        key = (j_e * 128 + p_e) * NCH + ch_e
        eo = np.argsort(key, kind="stable")
        key_s = key[eo]
        r_s = r_e[eo]
        uk, firsts = np.unique(key_s, return_index=True)
        dpos = np.arange(len(key_s)) - firsts[np.searchsorted(uk, key_s)]
        j_s = j_e[eo]
        p_s = p_e[eo]
        ch_s = ch_e[eo]

        # per-core per-block counts (for K and mask)
        cblk = np.zeros((N_BLOCKS, 128, NCH), np.int32)
        gb = 8 * np.arange(N_BLOCKS) + c
        cblk[:] = cnt_sorted.reshape(N_GBLK, 128, NCH)[gb]
        bmc = cblk.max(axis=1)  # [98, 4]
        K = np.minimum(np.maximum(bmc, (W > 0).astype(np.int32)), W)

        idxs = np.full((N_BLOCKS * NCH, ), None, dtype=object)
        counts = []
        for j in range(N_BLOCKS):
            for ch in range(NCH):
                w = int(W[j, ch])
                if w == 0:
                    continue
                k = int(K[j, ch])
                arr = np.full((w, 128), -1, np.int16)
                arr[:k, :] = 0
                idxs[j * NCH + ch] = arr
                counts.append(128 * k)
        sel = (j_s * NCH + ch_s).astype(np.int64)
        # place all edges vectorized-ish
        for cell in np.unique(sel):
            mm = sel == cell
            idxs[cell][dpos[mm], p_s[mm]] = r_s[mm]
        cols = []
        for cell in range(N_BLOCKS * NCH):
            a = idxs[cell]
            if a is None:
                continue
            flat = a.reshape(-1)
            wrap = flat.reshape(-1, 16).T
            cols.append(np.tile(wrap, (8, 1)))
        idx_mat = np.concatenate(cols, axis=1).astype(np.int16)
        assert idx_mat.shape == (128, IDX_COLS), idx_mat.shape
        counts_v = np.array(counts, np.int32)[None, :]

        # mask [128, MASK_COLS] bf16 (1.0/0.0)
        mcols = []
        for j in range(N_BLOCKS):
            for ch in range(NCH):
                w = int(W[j, ch])
                if w == 0:
                    continue
                mk = (np.arange(w)[None, :] < cblk[j, :, ch][:, None])
                mcols.append(mk)
        mask = np.concatenate(mcols, axis=1).astype(np.float32)
        mask_bf = to_bf16_bytes(mask)

        # xTperm [128, 12544] bf16: x columns of core targets in slot order
        kidx = (8 * np.arange(N_BLOCKS)[:, None] + c) * 128 + np.arange(128)[None, :]
        kidx = kidx.reshape(-1)  # [12544] global sorted rank
        gid = np.full(N_BLOCKS * 128, -1, np.int64)
        real = kidx < N_NODES
        gid[real] = order[kidx[real]]
        xp = np.zeros((IN_F, N_BLOCKS * 128), np.float32)
        xp[:, real] = x[gid[real]].T
        xperm_bf = to_bf16_bytes(xp)

        per_core.append(dict(idx=idx_mat, counts=counts_v, mask=mask_bf,
                             xperm=xperm_bf, gid=gid, real=real))

    common = dict(W=W, Wmax=Wmax, IDX_COLS=IDX_COLS, MASK_COLS=MASK_COLS,
                  NCALLS=len(per_core[0]["counts"][0]),
                  xT=xT_bf, pack0=pack0_bf, pack2=pack2_bf)
    return common, per_core


def _build_program(W, Wmax, IDX_COLS, MASK_COLS, NCALLS):
    nc = bacc.Bacc("TRN2", debug=False, num_devices=N_CORES,
                   num_swdge_queues=4)
    f32 = mybir.dt.float32
    bf16 = mybir.dt.bfloat16
    i16 = mybir.dt.int16
    i32 = mybir.dt.int32
    u16 = mybir.dt.uint16

    Wsum = W.sum(axis=1)

    xT_d = nc.dram_tensor("xT", [IN_F, TABLE_ROWS], u16,
                          kind="ExternalInput").ap()
    xperm_d = nc.dram_tensor("xperm", [IN_F, N_BLOCKS * 128], u16,
                             kind="ExternalInput").ap()
    pack0_d = nc.dram_tensor("pack0", [IN_F, HF + H], u16,
                             kind="ExternalInput").ap()
    pack2_d = nc.dram_tensor("pack2", [IN_F, HF + H], u16,
                             kind="ExternalInput").ap()
    idx_d = nc.dram_tensor("idx", [128, IDX_COLS], i16,
                           kind="ExternalInput").ap()
    cnts_d = nc.dram_tensor("cnts", [1, NCALLS], i32,
                            kind="ExternalInput").ap()
    mask_d = nc.dram_tensor("mask", [128, MASK_COLS], u16,
                            kind="ExternalInput").ap()
    out_d = nc.dram_tensor("out", [128 * N_BLOCKS, HF], f32,
                           kind="ExternalOutput").ap()
    table = nc.dram_tensor("table", [TABLE_ROWS, ROW_E], bf16).ap()

    with tile.TileContext(nc) as tc, ExitStack() as ctx:
        consts = ctx.enter_context(tc.tile_pool(name="consts", bufs=1))
        bstg = ctx.enter_context(tc.tile_pool(name="bstg", bufs=2))
        rowp = ctx.enter_context(tc.tile_pool(name="rowp", bufs=2))
        idxp = ctx.enter_context(tc.tile_pool(name="idxp", bufs=2))
        xpp = ctx.enter_context(tc.tile_pool(name="xpp", bufs=2))
        gpool = ctx.enter_context(tc.tile_pool(name="gpool", bufs=G_BUFS))
        ework = ctx.enter_context(tc.tile_pool(name="ework", bufs=2))
        mwork = ctx.enter_context(tc.tile_pool(name="mwork", bufs=1))
        twork = ctx.enter_context(tc.tile_pool(name="twork", bufs=1))
        outp = ctx.enter_context(tc.tile_pool(name="outp", bufs=2))
        psA = ctx.enter_context(tc.tile_pool(name="psA", bufs=2, space="PSUM"))
        psB = ctx.enter_context(tc.tile_pool(name="psB", bufs=3, space="PSUM"))

        nc.gpsimd.load_library(mlp)

        # ---- consts ----
        pk0 = consts.tile([IN_F, HF + H], bf16)
        nc.sync.dma_start(out=pk0[:].bitcast(mybir.dt.uint16), in_=pack0_d[:])
        pk2 = consts.tile([IN_F, HF + H], bf16)
        nc.sync.dma_start(out=pk2[:].bitcast(mybir.dt.uint16), in_=pack2_d[:])
        mask_t = consts.tile([128, MASK_COLS], bf16)
        nc.sync.dma_start(out=mask_t[:].bitcast(mybir.dt.uint16), in_=mask_d[:])
        cnt_t = consts.tile([1, NCALLS], i32)
        nc.sync.dma_start(out=cnt_t[:], in_=cnts_d[:])

        # pre-zero G buffers (avoid NaN garbage in skipped gather slots)
        g_tiles = []
        for _ in range(G_BUFS):
            g = gpool.tile([128, Wmax, ROW_E], bf16, tag="G")
            nc.gpsimd.memset(g[:], 0.0)
            g_tiles.append(g)

        # ---- phase B: projection table (partition-major rows) ----
        for ch in range(NCH):
            tch = T_CH[ch]
            groups = [(g0, min(14, tch - g0)) for g0 in range(0, tch, 14)]
            for g0, gn in groups:
                xtb = bstg.tile([IN_F, 14 * 128], bf16, tag="xtb")
                c0 = CH_BASE_R[ch] + g0 * 128
                nc.sync.dma_start(
                    out=xtb[:, :gn * 128].bitcast(mybir.dt.uint16),
                    in_=xT_d[:, c0:c0 + gn * 128])
                rowb = rowp.tile([128, 14, ROW_E], bf16, tag="rowb")
                for t in range(gn):
                    pa = psA.tile([128, HF + H], f32, space="PSUM", tag="pa")
                    nc.tensor.matmul(out=pa[:],
                                     lhsT=xtb[:, t * 128:(t + 1) * 128],
                                     rhs=pk0[:], start=True, stop=True)
                    nc.scalar.activation(
                        out=rowb[:, t, 0:HF + H], in_=pa[:],
                        func=mybir.ActivationFunctionType.Copy)
                tv = table[CH_BASE_R[ch]:
                           CH_BASE_R[ch] + CH_ROWS[ch], :].rearrange(
                               "(p t) e -> p t e", t=tch)
                nc.sync.dma_start(out=tv[:, g0:g0 + gn, :],
                                  in_=rowb[:, :gn, :])

        # ---- phase C ----
        col_off = np.zeros(N_BLOCKS * NCH + 1, np.int64)
        np.cumsum(8 * W.reshape(-1), out=col_off[1:])
        moff_all = np.zeros(N_BLOCKS + 1, np.int64)
        np.cumsum(Wsum, out=moff_all[1:])

        regs = [nc.gpsimd.alloc_register(f"gc{i}") for i in range(4)]
        call_i = 0
        IDX_GRP = 8
        XP_GRP = 8
        OUT_GRP = 7
        outb = None
        for j in range(N_BLOCKS):
            wt = int(Wsum[j])
            if j % IDX_GRP == 0:
                g0c = int(col_off[j * NCH])
                g1c = int(col_off[min(j + IDX_GRP, N_BLOCKS) * NCH])
                idx_t = idxp.tile([128, g1c - g0c], i16, tag="idxg")
                nc.sync.dma_start(out=idx_t[:], in_=idx_d[:, g0c:g1c])
                idx_base = g0c
            if j % XP_GRP == 0:
                nxp = min(XP_GRP, N_BLOCKS - j)
                xpt = xpp.tile([IN_F, XP_GRP * 128], bf16, tag="xpt")
                nc.sync.dma_start(
                    out=xpt[:, :nxp * 128].bitcast(mybir.dt.uint16),
                    in_=xperm_d[:, j * 128:(j + nxp) * 128])
                xp_base = j
            if j % OUT_GRP == 0:
                outb = outp.tile([128, OUT_GRP, HF], f32, tag="outb")

            sk = psB.tile([128, HF + H], f32, space="PSUM", tag="sk")
            xo = (j - xp_base) * 128
            nc.tensor.matmul(out=sk[:], lhsT=xpt[:, xo:xo + 128], rhs=pk2[:],
                             start=True, stop=True)
            beta = ework.tile([128, H], bf16, tag="beta")
            nc.vector.tensor_copy(out=beta[:], in_=sk[:, HF:HF + H])

            if wt > 0:
                G = gpool.tile([128, Wmax, ROW_E], bf16, tag="G")
                doff = 0
                for ch in range(NCH):
                    w = int(W[j, ch])
                    if w == 0:
                        continue
                    c0 = int(col_off[j * NCH + ch]) - idx_base
                    q = ch
                    nc.gpsimd.reg_load(regs[q], cnt_t[0:1, call_i:call_i + 1])
                    tab_ch = table[CH_BASE_R[ch]:CH_BASE_R[ch] + CH_ROWS[ch], :]
                    nc.gpsimd.dma_gather(
                        G[:, doff:doff + w, :], tab_ch,
                        idx_t[:, c0:c0 + 8 * w],
                        128 * w, regs[q], ROW_E,
                        single_packet=SINGLE_PACKET, queue_num=q)
                    call_i += 1
                    doff += w

                mo = int(moff_all[j])
                DW = HF + H  # 136: [weighted p | E] fused row
                s_t = ework.tile([128, Wmax, H], f32, tag="s")
                nc.vector.tensor_tensor(
                    out=s_t[:, :wt], in0=G[:, :wt, HF:HF + H],
                    in1=beta[:].unsqueeze(1).to_broadcast([128, wt, H]),
                    op=mybir.AluOpType.add)
                e1 = ework.tile([128, Wmax, H], f32, tag="e1")
                nc.scalar.activation(out=e1[:, :wt], in_=s_t[:, :wt],
                                     func=mybir.ActivationFunctionType.Exp,
                                     scale=NEG_SLOPE)
                r_t = ework.tile([128, Wmax, H], f32, tag="rt")
                nc.scalar.activation(out=r_t[:, :wt], in_=s_t[:, :wt],
                                     func=mybir.ActivationFunctionType.Relu)
                e2 = ework.tile([128, Wmax, H], f32, tag="e2")
                nc.scalar.activation(out=e2[:, :wt], in_=r_t[:, :wt],
                                     func=mybir.ActivationFunctionType.Exp,
                                     scale=1.0 - NEG_SLOPE)
                eb = ework.tile([128, Wmax, H], bf16, tag="eb")
                nc.vector.tensor_tensor(out=eb[:, :wt], in0=e1[:, :wt],
                                        in1=e2[:, :wt],
                                        op=mybir.AluOpType.mult)
                # mp rows: [0:128] = p * E (bcast over f), [128:136] = E*mask
                mp = mwork.tile([128, Wmax, DW], bf16, tag="mp")
                nc.vector.tensor_tensor(
                    out=mp[:, :wt, HF:HF + H], in0=eb[:, :wt],
                    in1=mask_t[:, mo:mo + wt].unsqueeze(2).to_broadcast(
                        [128, wt, H]),
                    op=mybir.AluOpType.mult)
                nc.vector.tensor_tensor(
                    out=mp[:, :wt, 0:HF].rearrange("p w (h f) -> p w h f",
                                                   h=H),
                    in0=G[:, :wt, 0:HF].rearrange("p w (h f) -> p w h f",
                                                  h=H),
                    in1=mp[:, :wt, HF:HF + H].unsqueeze(3).to_broadcast(
                        [128, wt, H, F]),
                    op=mybir.AluOpType.mult)

                # bf16 tree reduce over w on [128, w, 136]; final level f32
                HT = (Wmax + 1) // 2
                tA = twork.tile([128, HT, DW], bf16, tag="tA")
                tB = twork.tile([128, HT, DW], bf16, tag="tB")
                tF = twork.tile([128, DW], f32, tag="tF")

                t = wt
                h = t // 2
                r = t - 2 * h
                if t == 1:
                    nc.vector.tensor_copy(out=tF[:], in_=mp[:, 0])
                else:
                    cur, nxt = mp, tA
                    while t > 2:
                        h = t // 2
                        r = t - 2 * h
                        nc.vector.tensor_tensor(out=nxt[:, 0:h],
                                                in0=cur[:, 0:h],
                                                in1=cur[:, h:2 * h],
                                                op=mybir.AluOpType.add)
                        if r:
                            nc.vector.tensor_copy(
                                out=nxt[:, h:h + 1],
                                in_=cur[:, 2 * h:2 * h + 1])
                        t = h + r
                        cur = nxt
                        nxt = tB if cur is tA else tA
                    nc.vector.tensor_tensor(out=tF[:], in0=cur[:, 0],
                                            in1=cur[:, 1],
                                            op=mybir.AluOpType.add)

                dinv = ework.tile([128, H], f32, tag="dinv")
                nc.vector.tensor_scalar_add(dinv[:], tF[:, HF:HF + H], EPS)
                nc.vector.reciprocal(out=dinv[:], in_=dinv[:])
                O = ework.tile([128, HF], f32, tag="O")
                nc.vector.tensor_tensor(
                    out=O[:].rearrange("p (h f) -> p h f", h=H),
                    in0=tF[:, 0:HF].rearrange("p (h f) -> p h f", h=H),
                    in1=dinv[:].unsqueeze(2).to_broadcast([128, H, F]),
                    op=mybir.AluOpType.mult)
                nc.vector.tensor_tensor(out=O[:], in0=O[:], in1=sk[:, 0:HF],
                                        op=mybir.AluOpType.add)
            else:
                O = ework.tile([128, HF], f32, tag="O")
                nc.vector.tensor_copy(out=O[:], in_=sk[:, 0:HF])

            A_t = ework.tile([128, HF], f32, tag="At")
            nc.vector.tensor_scalar(out=A_t[:], in0=O[:], scalar1=0.0,
                                    scalar2=-1.0, op0=mybir.AluOpType.max,
                                    op1=mybir.AluOpType.add)
            T_t = ework.tile([128, HF], f32, tag="Tt")
            nc.vector.tensor_scalar_min(T_t[:], O[:], 0.0)
            E2 = ework.tile([128, HF], f32, tag="E2")
            nc.scalar.activation(out=E2[:], in_=T_t[:],
                                 func=mybir.ActivationFunctionType.Exp)
            jo = j % OUT_GRP
            nc.vector.tensor_tensor(out=outb[:, jo], in0=A_t[:], in1=E2[:],
                                    op=mybir.AluOpType.add)
            if jo == OUT_GRP - 1 or j == N_BLOCKS - 1:
                j0 = j - jo
                ov = out_d.rearrange("(p j) e -> p j e", j=N_BLOCKS)
                nc.sync.dma_start(out=ov[:, j0:j + 1, :],
                                  in_=outb[:, :jo + 1, :])

    nc.compile()
    return nc


def kernel(x, edge_index, W_proj, W_skip, a_src, a_tgt):
    common, per_core = _host_prep(x, edge_index, W_proj, W_skip, a_src, a_tgt)
    key = common["W"].tobytes()
    if key not in _COMPILED:
        _COMPILED[key] = _build_program(
            common["W"], common["Wmax"], common["IDX_COLS"],
            common["MASK_COLS"], common["NCALLS"])
    nc = _COMPILED[key]

    in_maps = []
    for c in range(N_CORES):
        pc = per_core[c]
        in_maps.append({
            "xT": common["xT"],
            "xperm": pc["xperm"],
            "pack0": common["pack0"],
            "pack2": common["pack2"],
            "idx": pc["idx"],
            "cnts": pc["counts"],
            "mask": pc["mask"],
        })
    trace = bool(int(os.environ.get("GAT_TRACE", "0")))
    res = run_bass_kernel_spmd(nc, in_maps, list(range(N_CORES)),
                               trace=trace)
    if trace:
        kernel.last_exec_time_ns = res.exec_time_ns
        kernel.last_mean_exec_time_ns = res.mean_exec_time_ns

    out = np.empty((N_NODES, HF), np.float32)
    for c in range(N_CORES):
        o = res.results[c]["out"].reshape(128, N_BLOCKS, HF)
        pc = per_core[c]
        gid = pc["gid"]
        real = pc["real"]
        # slot order: k = j*128+p maps o[p, j]
        o_pj = np.transpose(o, (1, 0, 2)).reshape(N_BLOCKS * 128, HF)
        out[gid[real]] = o_pj[real]
    return out


kernel.last_exec_time_ns = None
kernel.last_mean_exec_time_ns = None


# revision 4
# speedup vs baseline: 1.0708x; 1.0212x over previous
"""GAT layer on 8 Trainium2 NeuronCores (Bass/Tile) — v2.

Target-per-partition layout (like baseline) with:
  - global count-sorted target clustering dealt round-robin to cores
    (shared static W = max over 8 cores per slot),
  - per-core runtime gather counts (trailing -1 idxs skipped by the
    SWDGE ucode) => descriptor count ~1.35x E instead of 2.06x,
  - bf16 phase B with fat (multi-KB) DMA descriptors via a
    partition-major table layout,
  - mask-based padding (no sentinel rows),
  - DVE tree-reduce instead of transposed tensor_reduce,
  - 4D inner-broadcast tensor_tensor (no Ex materialization).
"""

import os
import sys

sys.path.insert(0, "/opt/trn_rl_repo")

import numpy as np
from contextlib import ExitStack

import concourse.bass as bass
import concourse.bacc as bacc
import concourse.tile as tile
from concourse import mybir
from concourse.bass_utils import run_bass_kernel_spmd
from concourse.library_config import mlp

N_NODES = 100000
N_EDGES = 1600000
IN_F = 128
H = 8
F = 16
HF = H * F
NEG_SLOPE = 0.2
EPS = 1e-16
N_CORES = 8
N_BLOCKS = 98
N_GBLK = 784  # global blocks of 128 sorted targets
NCH = 4
CH_NODES = [25088, 25088, 25088, 24736]
T_CH = [196, 196, 196, 194]
CH_BASE_N = [0, 25088, 50176, 75264]
CH_ROWS = [128 * t for t in T_CH]  # 25088,25088,25088,24832
CH_BASE_R = [0, 25088, 50176, 75264]
TABLE_ROWS = CH_BASE_R[-1] + CH_ROWS[-1]  # 100096
ROW_E = 256  # bf16 elems per table row (512B)
XT_COLS = 100352  # 784*128, padded
ALPHA_PAD = -300.0
G_BUFS = 3
SINGLE_PACKET = False

_COMPILED = {}


def _host_prep(x, edge_index, W_proj, W_skip, a_src, a_tgt):
    x = np.asarray(x, np.float32)
    ei = np.asarray(edge_index).astype(np.int64)
    src, tgt = ei[0], ei[1]

    ch_of = np.minimum(src // 25088, 3)
    l = src - np.take(CH_BASE_N, ch_of)
    row_of = (l % 128) * np.take(T_CH, ch_of) + l // 128  # local row in chunk

    cnt = np.zeros((N_NODES, NCH), np.int32)
    np.add.at(cnt, (tgt, ch_of), 1)
    order = np.lexsort((-cnt[:, 3], -cnt[:, 2], -cnt[:, 1], -cnt[:, 0]))
    rank = np.empty(N_NODES, np.int64)
    rank[order] = np.arange(N_NODES)
    blk = rank // 128  # global block of each target
    pos = rank % 128
    core_of = blk % 8
    slot_of = blk // 8

    cnt_sorted = np.zeros((N_GBLK * 128, NCH), np.int32)
    cnt_sorted[:N_NODES] = cnt[order]
    bm = cnt_sorted.reshape(N_GBLK, 128, NCH).max(axis=1)  # per global block
    W = bm.reshape(N_BLOCKS, 8, NCH).max(axis=1).astype(np.int32)
    Wsum = W.sum(axis=1)
    Wmax = int(Wsum.max())

    # weight packs (host-side repack of params only)
    Ablk = np.zeros((HF, H), np.float32)
    Bblk = np.zeros((HF, H), np.float32)
    asr = np.asarray(a_src, np.float32).reshape(H, F)
    atg = np.asarray(a_tgt, np.float32).reshape(H, F)
    for h in range(H):
        Ablk[h * F:(h + 1) * F, h] = asr[h]
        Bblk[h * F:(h + 1) * F, h] = atg[h]
    Wp = np.asarray(W_proj, np.float32)
    pack0 = np.concatenate([Wp, Wp @ Ablk], axis=1)  # [128,136]
    pack2 = np.concatenate([np.asarray(W_skip, np.float32), Wp @ Bblk], axis=1)

    import jax.numpy as jnp

    def to_bf16_bytes(a):
        return np.asarray(jnp.asarray(a, jnp.bfloat16)).view(np.uint16)

    xT = np.zeros((IN_F, XT_COLS), np.float32)
    xT[:, :N_NODES] = x.T
    xT_bf = to_bf16_bytes(xT[:, :TABLE_ROWS])  # device input [128, 100096] u16
    pack0_bf = to_bf16_bytes(pack0)
    pack2_bf = to_bf16_bytes(pack2)

    # per-(core, slot, chunk) edge placement
    e_core = core_of[tgt]
    e_slot = slot_of[tgt]
    e_pos = pos[tgt]
    e_ch = ch_of

    col_off = np.zeros(N_BLOCKS * NCH + 1, np.int64)  # idx col offsets (8*W)
    np.cumsum(8 * W.reshape(-1), out=col_off[1:])
    IDX_COLS = int(col_off[-1])
    MASK_COLS = int(Wsum.sum())

    per_core = []
    for c in range(N_CORES):
        m = e_core == c
        j_e = e_slot[m]
        p_e = e_pos[m]
        ch_e = e_ch[m]
        r_e = row_of[m]
        key = (j_e * 128 + p_e) * NCH + ch_e
        eo = np.argsort(key, kind="stable")
        key_s = key[eo]
        r_s = r_e[eo]
        uk, firsts = np.unique(key_s, return_index=True)
        dpos = np.arange(len(key_s)) - firsts[np.searchsorted(uk, key_s)]
        j_s = j_e[eo]
        p_s = p_e[eo]
        ch_s = ch_e[eo]

        # per-core per-block counts (for K and mask)
        cblk = np.zeros((N_BLOCKS, 128, NCH), np.int32)
        gb = 8 * np.arange(N_BLOCKS) + c
        cblk[:] = cnt_sorted.reshape(N_GBLK, 128, NCH)[gb]
        bmc = cblk.max(axis=1)  # [98, 4]
        K = np.minimum(np.maximum(bmc, (W > 0).astype(np.int32)), W)

        idxs = np.full((N_BLOCKS * NCH, ), None, dtype=object)
        counts = []
        for j in range(N_BLOCKS):
            for ch in range(NCH):
                w = int(W[j, ch])
                if w == 0:
                    continue
                k = int(K[j, ch])
                arr = np.full((w, 128), -1, np.int16)
                arr[:k, :] = 0
                idxs[j * NCH + ch] = arr
                counts.append(128 * k)
        sel = (j_s * NCH + ch_s).astype(np.int64)
        # place all edges vectorized-ish
        for cell in np.unique(sel):
            mm = sel == cell
            idxs[cell][dpos[mm], p_s[mm]] = r_s[mm]
        cols = []
        for cell in range(N_BLOCKS * NCH):
            a = idxs[cell]
            if a is None:
                continue
            flat = a.reshape(-1)
            wrap = flat.reshape(-1, 16).T
            cols.append(np.tile(wrap, (8, 1)))
        idx_mat = np.concatenate(cols, axis=1).astype(np.int16)
        assert idx_mat.shape == (128, IDX_COLS), idx_mat.shape
        counts_v = np.array(counts, np.int32)[None, :]

        # mask [128, MASK_COLS] bf16 (1.0/0.0)
        mcols = []
        for j in range(N_BLOCKS):
            for ch in range(NCH):
                w = int(W[j, ch])
                if w == 0:
                    continue
                mk = (np.arange(w)[None, :] < cblk[j, :, ch][:, None])
                mcols.append(mk)
        mask = np.concatenate(mcols, axis=1).astype(np.float32)
        mask_bf = to_bf16_bytes(mask)

        # xTperm [128, 12544] bf16: x columns of core targets in slot order
        kidx = (8 * np.arange(N_BLOCKS)[:, None] + c) * 128 + np.arange(128)[None, :]
        kidx = kidx.reshape(-1)  # [12544] global sorted rank
        gid = np.full(N_BLOCKS * 128, -1, np.int64)
        real = kidx < N_NODES
        gid[real] = order[kidx[real]]
        xp = np.zeros((IN_F, N_BLOCKS * 128), np.float32)
        xp[:, real] = x[gid[real]].T
        xperm_bf = to_bf16_bytes(xp)

        per_core.append(dict(idx=idx_mat, counts=counts_v, mask=mask_bf,
                             xperm=xperm_bf, gid=gid, real=real))

    common = dict(W=W, Wmax=Wmax, IDX_COLS=IDX_COLS, MASK_COLS=MASK_COLS,
                  NCALLS=len(per_core[0]["counts"][0]),
                  xT=xT_bf, pack0=pack0_bf, pack2=pack2_bf)
    return common, per_core


def _build_program(W, Wmax, IDX_COLS, MASK_COLS, NCALLS):
    nc = bacc.Bacc("TRN2", debug=False, num_devices=N_CORES,
                   num_swdge_queues=4)
    f32 = mybir.dt.float32
    bf16 = mybir.dt.bfloat16
    i16 = mybir.dt.int16
    i32 = mybir.dt.int32
    u16 = mybir.dt.uint16

    Wsum = W.sum(axis=1)

    xT_d = nc.dram_tensor("xT", [IN_F, TABLE_ROWS], u16,
                          kind="ExternalInput").ap()
    xperm_d = nc.dram_tensor("xperm", [IN_F, N_BLOCKS * 128], u16,
                             kind="ExternalInput").ap()
    pack0_d = nc.dram_tensor("pack0", [IN_F, HF + H], u16,
                             kind="ExternalInput").ap()
    pack2_d = nc.dram_tensor("pack2", [IN_F, HF + H], u16,
                             kind="ExternalInput").ap()
    idx_d = nc.dram_tensor("idx", [128, IDX_COLS], i16,
                           kind="ExternalInput").ap()
    cnts_d = nc.dram_tensor("cnts", [1, NCALLS], i32,
                            kind="ExternalInput").ap()
    mask_d = nc.dram_tensor("mask", [128, MASK_COLS], u16,
                            kind="ExternalInput").ap()
    out_d = nc.dram_tensor("out", [128 * N_BLOCKS, HF], f32,
                           kind="ExternalOutput").ap()
    tables = [nc.dram_tensor(f"table{c}", [CH_ROWS[c], ROW_E], bf16).ap()
              for c in range(NCH)]

    with tile.TileContext(nc) as tc, ExitStack() as ctx:
        consts = ctx.enter_context(tc.tile_pool(name="consts", bufs=1))
        bstg = ctx.enter_context(tc.tile_pool(name="bstg", bufs=2))
        rowp = ctx.enter_context(tc.tile_pool(name="rowp", bufs=2))
        idxp = ctx.enter_context(tc.tile_pool(name="idxp", bufs=2))
        xpp = ctx.enter_context(tc.tile_pool(name="xpp", bufs=2))
        gpool = ctx.enter_context(tc.tile_pool(name="gpool", bufs=G_BUFS))
        ework = ctx.enter_context(tc.tile_pool(name="ework", bufs=2))
        mwork = ctx.enter_context(tc.tile_pool(name="mwork", bufs=1))
        twork = ctx.enter_context(tc.tile_pool(name="twork", bufs=1))
        outp = ctx.enter_context(tc.tile_pool(name="outp", bufs=2))
        psA = ctx.enter_context(tc.tile_pool(name="psA", bufs=2, space="PSUM"))
        psB = ctx.enter_context(tc.tile_pool(name="psB", bufs=3, space="PSUM"))

        nc.gpsimd.load_library(mlp)

        # ---- consts ----
        pk0 = consts.tile([IN_F, HF + H], bf16)
        nc.sync.dma_start(out=pk0[:].bitcast(mybir.dt.uint16), in_=pack0_d[:])
        pk2 = consts.tile([IN_F, HF + H], bf16)
        nc.sync.dma_start(out=pk2[:].bitcast(mybir.dt.uint16), in_=pack2_d[:])
        mask_t = consts.tile([128, MASK_COLS], bf16)
        nc.sync.dma_start(out=mask_t[:].bitcast(mybir.dt.uint16), in_=mask_d[:])
        cnt_t = consts.tile([1, NCALLS], i32)
        nc.sync.dma_start(out=cnt_t[:], in_=cnts_d[:])

        # pre-zero G buffers (avoid NaN garbage in skipped gather slots)
        g_tiles = []
        for _ in range(G_BUFS):
            g = gpool.tile([128, Wmax, ROW_E], bf16, tag="G")
            nc.gpsimd.memset(g[:], 0.0)
            g_tiles.append(g)

        # ---- phase B: projection table (partition-major rows) ----
        for ch in range(NCH):
            tch = T_CH[ch]
            groups = [(g0, min(14, tch - g0)) for g0 in range(0, tch, 14)]
            for g0, gn in groups:
                xtb = bstg.tile([IN_F, 14 * 128], bf16, tag="xtb")
                c0 = CH_BASE_R[ch] + g0 * 128
                nc.sync.dma_start(
                    out=xtb[:, :gn * 128].bitcast(mybir.dt.uint16),
                    in_=xT_d[:, c0:c0 + gn * 128])
                rowb = rowp.tile([128, 14, ROW_E], bf16, tag="rowb")
                for t in range(gn):
                    pa = psA.tile([128, HF + H], f32, space="PSUM", tag="pa")
                    nc.tensor.matmul(out=pa[:],
                                     lhsT=xtb[:, t * 128:(t + 1) * 128],
                                     rhs=pk0[:], start=True, stop=True)
                    nc.scalar.activation(
                        out=rowb[:, t, 0:HF + H], in_=pa[:],
                        func=mybir.ActivationFunctionType.Copy)
                tv = tables[ch].rearrange("(p t) e -> p t e", t=tch)
                nc.sync.dma_start(out=tv[:, g0:g0 + gn, :],
                                  in_=rowb[:, :gn, :])

        # ---- phase C ----
        col_off = np.zeros(N_BLOCKS * NCH + 1, np.int64)
        np.cumsum(8 * W.reshape(-1), out=col_off[1:])
        moff_all = np.zeros(N_BLOCKS + 1, np.int64)
        np.cumsum(Wsum, out=moff_all[1:])

        regs = [nc.gpsimd.alloc_register(f"gc{i}") for i in range(4)]
        call_i = 0
        IDX_GRP = 8
        XP_GRP = 8
        OUT_GRP = 7
        outb = None
        for j in range(N_BLOCKS):
            wt = int(Wsum[j])
            if j % IDX_GRP == 0:
                g0c = int(col_off[j * NCH])
                g1c = int(col_off[min(j + IDX_GRP, N_BLOCKS) * NCH])
                idx_t = idxp.tile([128, g1c - g0c], i16, tag="idxg")
                nc.sync.dma_start(out=idx_t[:], in_=idx_d[:, g0c:g1c])
                idx_base = g0c
            if j % XP_GRP == 0:
                nxp = min(XP_GRP, N_BLOCKS - j)
                xpt = xpp.tile([IN_F, XP_GRP * 128], bf16, tag="xpt")
                nc.sync.dma_start(
                    out=xpt[:, :nxp * 128].bitcast(mybir.dt.uint16),
                    in_=xperm_d[:, j * 128:(j + nxp) * 128])
                xp_base = j
            if j % OUT_GRP == 0:
                outb = outp.tile([128, OUT_GRP, HF], f32, tag="outb")

            sk = psB.tile([128, HF + H], f32, space="PSUM", tag="sk")
            xo = (j - xp_base) * 128
            nc.tensor.matmul(out=sk[:], lhsT=xpt[:, xo:xo + 128], rhs=pk2[:],
                             start=True, stop=True)
            beta = ework.tile([128, H], bf16, tag="beta")
            nc.vector.tensor_copy(out=beta[:], in_=sk[:, HF:HF + H])

            if wt > 0:
                G = gpool.tile([128, Wmax, ROW_E], bf16, tag="G")
                doff = 0
                for ch in range(NCH):
                    w = int(W[j, ch])
                    if w == 0:
                        continue
                    c0 = int(col_off[j * NCH + ch]) - idx_base
                    q = ch
                    nc.gpsimd.reg_load(regs[q], cnt_t[0:1, call_i:call_i + 1])
                    tab_ch = tables[ch]
                    nc.gpsimd.dma_gather(
                        G[:, doff:doff + w, :], tab_ch,
                        idx_t[:, c0:c0 + 8 * w],
                        128 * w, regs[q], ROW_E,
                        single_packet=SINGLE_PACKET, queue_num=q)
                    call_i += 1
                    doff += w

                mo = int(moff_all[j])
                DW = HF + H  # 136: [weighted p | E] fused row
                s_t = ework.tile([128, Wmax, H], f32, tag="s")
                nc.vector.tensor_tensor(
                    out=s_t[:, :wt], in0=G[:, :wt, HF:HF + H],
                    in1=beta[:].unsqueeze(1).to_broadcast([128, wt, H]),
                    op=mybir.AluOpType.add)
                e1 = ework.tile([128, Wmax, H], f32, tag="e1")
                nc.scalar.activation(out=e1[:, :wt], in_=s_t[:, :wt],
                                     func=mybir.ActivationFunctionType.Exp,
                                     scale=NEG_SLOPE)
                r_t = ework.tile([128, Wmax, H], f32, tag="rt")
                nc.scalar.activation(out=r_t[:, :wt], in_=s_t[:, :wt],
                                     func=mybir.ActivationFunctionType.Relu)
                e2 = ework.tile([128, Wmax, H], f32, tag="e2")
                nc.scalar.activation(out=e2[:, :wt], in_=r_t[:, :wt],
                                     func=mybir.ActivationFunctionType.Exp,
                                     scale=1.0 - NEG_SLOPE)
                eb = ework.tile([128, Wmax, H], bf16, tag="eb")
                nc.vector.tensor_tensor(out=eb[:, :wt], in0=e1[:, :wt],
                                        in1=e2[:, :wt],
                                        op=mybir.AluOpType.mult)
                # mp rows: [0:128] = p * E (bcast over f), [128:136] = E*mask
                mp = mwork.tile([128, Wmax, DW], bf16, tag="mp")
                nc.vector.tensor_tensor(
                    out=mp[:, :wt, HF:HF + H], in0=eb[:, :wt],
                    in1=mask_t[:, mo:mo + wt].unsqueeze(2).to_broadcast(
                        [128, wt, H]),
                    op=mybir.AluOpType.mult)
                nc.vector.tensor_tensor(
                    out=mp[:, :wt, 0:HF].rearrange("p w (h f) -> p w h f",
                                                   h=H),
                    in0=G[:, :wt, 0:HF].rearrange("p w (h f) -> p w h f",
                                                  h=H),
                    in1=mp[:, :wt, HF:HF + H].unsqueeze(3).to_broadcast(
                        [128, wt, H, F]),
                    op=mybir.AluOpType.mult)

                # bf16 tree reduce over w on [128, w, 136]; final level f32
                HT = (Wmax + 1) // 2
                tA = twork.tile([128, HT, DW], bf16, tag="tA")
                tB = twork.tile([128, HT, DW], bf16, tag="tB")
                tF = twork.tile([128, DW], f32, tag="tF")

                t = wt
                h = t // 2
                r = t - 2 * h
                if t == 1:
                    nc.vector.tensor_copy(out=tF[:], in_=mp[:, 0])
                else:
                    cur, nxt = mp, tA
                    while t > 2:
                        h = t // 2
                        r = t - 2 * h
                        nc.vector.tensor_tensor(out=nxt[:, 0:h],
                                                in0=cur[:, 0:h],
                                                in1=cur[:, h:2 * h],
                                                op=mybir.AluOpType.add)
                        if r:
                            nc.vector.tensor_copy(
                                out=nxt[:, h:h + 1],
                                in_=cur[:, 2 * h:2 * h + 1])
                        t = h + r
                        cur = nxt
                        nxt = tB if cur is tA else tA
                    nc.vector.tensor_tensor(out=tF[:], in0=cur[:, 0],
                                            in1=cur[:, 1],
                                            op=mybir.AluOpType.add)

                dinv = ework.tile([128, H], f32, tag="dinv")
                nc.vector.tensor_scalar_add(dinv[:], tF[:, HF:HF + H], EPS)
                nc.vector.reciprocal(out=dinv[:], in_=dinv[:])
                O = ework.tile([128, HF], f32, tag="O")
                nc.vector.tensor_tensor(
                    out=O[:].rearrange("p (h f) -> p h f", h=H),
                    in0=tF[:, 0:HF].rearrange("p (h f) -> p h f", h=H),
                    in1=dinv[:].unsqueeze(2).to_broadcast([128, H, F]),
                    op=mybir.AluOpType.mult)
                nc.vector.tensor_tensor(out=O[:], in0=O[:], in1=sk[:, 0:HF],
                                        op=mybir.AluOpType.add)
            else:
                O = ework.tile([128, HF], f32, tag="O")
                nc.vector.tensor_copy(out=O[:], in_=sk[:, 0:HF])

            A_t = ework.tile([128, HF], f32, tag="At")
            nc.vector.tensor_scalar(out=A_t[:], in0=O[:], scalar1=0.0,
                                    scalar2=-1.0, op0=mybir.AluOpType.max,
                                    op1=mybir.AluOpType.add)
            T_t = ework.tile([128, HF], f32, tag="Tt")
            nc.vector.tensor_scalar_min(T_t[:], O[:], 0.0)
            E2 = ework.tile([128, HF], f32, tag="E2")
            nc.scalar.activation(out=E2[:], in_=T_t[:],
                                 func=mybir.ActivationFunctionType.Exp)
            jo = j % OUT_GRP
            nc.vector.tensor_tensor(out=outb[:, jo], in0=A_t[:], in1=E2[:],
                                    op=mybir.AluOpType.add)
            if jo == OUT_GRP - 1 or j == N_BLOCKS - 1:
                j0 = j - jo
                ov = out_d.rearrange("(p j) e -> p j e", j=N_BLOCKS)
                nc.sync.dma_start(out=ov[:, j0:j + 1, :],
                                  in_=outb[:, :jo + 1, :])

    nc.compile()
    return nc


def kernel(x, edge_index, W_proj, W_skip, a_src, a_tgt):
    common, per_core = _host_prep(x, edge_index, W_proj, W_skip, a_src, a_tgt)
    key = common["W"].tobytes()
    if key not in _COMPILED:
        _COMPILED[key] = _build_program(
            common["W"], common["Wmax"], common["IDX_COLS"],
            common["MASK_COLS"], common["NCALLS"])
    nc = _COMPILED[key]

    in_maps = []
    for c in range(N_CORES):
        pc = per_core[c]
        in_maps.append({
            "xT": common["xT"],
            "xperm": pc["xperm"],
            "pack0": common["pack0"],
            "pack2": common["pack2"],
            "idx": pc["idx"],
            "cnts": pc["counts"],
            "mask": pc["mask"],
        })
    trace = bool(int(os.environ.get("GAT_TRACE", "0")))
    res = run_bass_kernel_spmd(nc, in_maps, list(range(N_CORES)),
                               trace=trace)
    if trace:
        kernel.last_exec_time_ns = res.exec_time_ns
        kernel.last_mean_exec_time_ns = res.mean_exec_time_ns

    out = np.empty((N_NODES, HF), np.float32)
    for c in range(N_CORES):
        o = res.results[c]["out"].reshape(128, N_BLOCKS, HF)
        pc = per_core[c]
        gid = pc["gid"]
        real = pc["real"]
        # slot order: k = j*128+p maps o[p, j]
        o_pj = np.transpose(o, (1, 0, 2)).reshape(N_BLOCKS * 128, HF)
        out[gid[real]] = o_pj[real]
    return out


kernel.last_exec_time_ns = None
kernel.last_mean_exec_time_ns = None


# revision 5
# speedup vs baseline: 1.1932x; 1.1143x over previous
"""GAT layer on 8 Trainium2 NeuronCores (Bass/Tile) — v2.

Target-per-partition layout (like baseline) with:
  - global count-sorted target clustering dealt round-robin to cores
    (shared static W = max over 8 cores per slot),
  - per-core runtime gather counts (trailing -1 idxs skipped by the
    SWDGE ucode) => descriptor count ~1.35x E instead of 2.06x,
  - bf16 phase B with fat (multi-KB) DMA descriptors via a
    partition-major table layout,
  - mask-based padding (no sentinel rows),
  - DVE tree-reduce instead of transposed tensor_reduce,
  - 4D inner-broadcast tensor_tensor (no Ex materialization).
"""

import os
import sys

sys.path.insert(0, "/opt/trn_rl_repo")

import numpy as np
from contextlib import ExitStack

import concourse.bass as bass
import concourse.bacc as bacc
import concourse.tile as tile
from concourse import mybir
from concourse.bass_utils import run_bass_kernel_spmd
from concourse.library_config import mlp

N_NODES = 100000
N_EDGES = 1600000
IN_F = 128
H = 8
F = 16
HF = H * F
NEG_SLOPE = 0.2
EPS = 1e-16
N_CORES = 8
N_BLOCKS = 98
N_GBLK = 784  # global blocks of 128 sorted targets
NCH = 4
CH_NODES = [25088, 25088, 25088, 24736]
T_CH = [196, 196, 196, 194]
CH_BASE_N = [0, 25088, 50176, 75264]
CH_ROWS = [128 * t for t in T_CH]  # 25088,25088,25088,24832
CH_BASE_R = [0, 25088, 50176, 75264]
TABLE_ROWS = CH_BASE_R[-1] + CH_ROWS[-1]  # 100096
ROW_E = 256  # bf16 elems per table row (512B)
XT_COLS = 100352  # 784*128, padded
ALPHA_PAD = -300.0
G_BUFS = 3
SINGLE_PACKET = False
DMA_SCRATCH = int(os.environ.get("GAT_SCRATCH", "32768"))

_COMPILED = {}


def _host_prep(x, edge_index, W_proj, W_skip, a_src, a_tgt):
    x = np.asarray(x, np.float32)
    ei = np.asarray(edge_index).astype(np.int64)
    src, tgt = ei[0], ei[1]

    ch_of = np.minimum(src // 25088, 3)
    l = src - np.take(CH_BASE_N, ch_of)
    row_of = (l % 128) * np.take(T_CH, ch_of) + l // 128  # local row in chunk

    cnt = np.zeros((N_NODES, NCH), np.int32)
    np.add.at(cnt, (tgt, ch_of), 1)
    order = np.lexsort((-cnt[:, 3], -cnt[:, 2], -cnt[:, 1], -cnt[:, 0]))
    rank = np.empty(N_NODES, np.int64)
    rank[order] = np.arange(N_NODES)
    blk = rank // 128  # global block of each target
    pos = rank % 128
    core_of = blk % 8
    slot_of = blk // 8

    cnt_sorted = np.zeros((N_GBLK * 128, NCH), np.int32)
    cnt_sorted[:N_NODES] = cnt[order]
    bm = cnt_sorted.reshape(N_GBLK, 128, NCH).max(axis=1)  # per global block
    W = bm.reshape(N_BLOCKS, 8, NCH).max(axis=1).astype(np.int32)
    Wsum = W.sum(axis=1)
    Wmax = int(Wsum.max())

    # weight packs (host-side repack of params only)
    Ablk = np.zeros((HF, H), np.float32)
    Bblk = np.zeros((HF, H), np.float32)
    asr = np.asarray(a_src, np.float32).reshape(H, F)
    atg = np.asarray(a_tgt, np.float32).reshape(H, F)
    for h in range(H):
        Ablk[h * F:(h + 1) * F, h] = asr[h]
        Bblk[h * F:(h + 1) * F, h] = atg[h]
    Wp = np.asarray(W_proj, np.float32)
    pack0 = np.concatenate([Wp, Wp @ Ablk], axis=1)  # [128,136]
    pack2 = np.concatenate([np.asarray(W_skip, np.float32), Wp @ Bblk], axis=1)

    import jax.numpy as jnp

    def to_bf16_bytes(a):
        return np.asarray(jnp.asarray(a, jnp.bfloat16)).view(np.uint16)

    xT = np.zeros((IN_F, XT_COLS), np.float32)
    xT[:, :N_NODES] = x.T
    xT_bf = to_bf16_bytes(xT[:, :TABLE_ROWS])  # device input [128, 100096] u16
    pack0_bf = to_bf16_bytes(pack0)
    pack2_bf = to_bf16_bytes(pack2)

    # per-(core, slot, chunk) edge placement
    e_core = core_of[tgt]
    e_slot = slot_of[tgt]
    e_pos = pos[tgt]
    e_ch = ch_of

    col_off = np.zeros(N_BLOCKS * NCH + 1, np.int64)  # idx col offsets (8*W)
    np.cumsum(8 * W.reshape(-1), out=col_off[1:])
    IDX_COLS = int(col_off[-1])
    MASK_COLS = int(Wsum.sum())

    per_core = []
    for c in range(N_CORES):
        m = e_core == c
        j_e = e_slot[m]
        p_e = e_pos[m]
        ch_e = e_ch[m]
        r_e = row_of[m]
        key = (j_e * 128 + p_e) * NCH + ch_e
        eo = np.argsort(key, kind="stable")
        key_s = key[eo]
        r_s = r_e[eo]
        uk, firsts = np.unique(key_s, return_index=True)
        dpos = np.arange(len(key_s)) - firsts[np.searchsorted(uk, key_s)]
        j_s = j_e[eo]
        p_s = p_e[eo]
        ch_s = ch_e[eo]

        # per-core per-block counts (for K and mask)
        cblk = np.zeros((N_BLOCKS, 128, NCH), np.int32)
        gb = 8 * np.arange(N_BLOCKS) + c
        cblk[:] = cnt_sorted.reshape(N_GBLK, 128, NCH)[gb]
        bmc = cblk.max(axis=1)  # [98, 4]
        K = np.minimum(np.maximum(bmc, (W > 0).astype(np.int32)), W)

        idxs = np.full((N_BLOCKS * NCH, ), None, dtype=object)
        counts = []
        for j in range(N_BLOCKS):
            for ch in range(NCH):
                w = int(W[j, ch])
                if w == 0:
                    continue
                k = int(K[j, ch])
                arr = np.full((w, 128), -1, np.int16)
                arr[:k, :] = 0
                idxs[j * NCH + ch] = arr
                counts.append(128 * k)
        sel = (j_s * NCH + ch_s).astype(np.int64)
        # place all edges vectorized-ish
        for cell in np.unique(sel):
            mm = sel == cell
            idxs[cell][dpos[mm], p_s[mm]] = r_s[mm]
        cols = []
        for cell in range(N_BLOCKS * NCH):
            a = idxs[cell]
            if a is None:
                continue
            flat = a.reshape(-1)
            wrap = flat.reshape(-1, 16).T
            cols.append(np.tile(wrap, (8, 1)))
        idx_mat = np.concatenate(cols, axis=1).astype(np.int16)
        assert idx_mat.shape == (128, IDX_COLS), idx_mat.shape
        counts_v = np.array(counts, np.int32)[None, :]

        # mask [128, MASK_COLS] bf16 (1.0/0.0)
        mcols = []
        for j in range(N_BLOCKS):
            for ch in range(NCH):
                w = int(W[j, ch])
                if w == 0:
                    continue
                mk = (np.arange(w)[None, :] < cblk[j, :, ch][:, None])
                mcols.append(mk)
        mask = np.concatenate(mcols, axis=1).astype(np.float32)
        mask_bf = to_bf16_bytes(mask)

        # xTperm [128, 12544] bf16: x columns of core targets in slot order
        kidx = (8 * np.arange(N_BLOCKS)[:, None] + c) * 128 + np.arange(128)[None, :]
        kidx = kidx.reshape(-1)  # [12544] global sorted rank
        gid = np.full(N_BLOCKS * 128, -1, np.int64)
        real = kidx < N_NODES
        gid[real] = order[kidx[real]]
        xp = np.zeros((IN_F, N_BLOCKS * 128), np.float32)
        xp[:, real] = x[gid[real]].T
        xperm_bf = to_bf16_bytes(xp)

        per_core.append(dict(idx=idx_mat, counts=counts_v, mask=mask_bf,
                             xperm=xperm_bf, gid=gid, real=real))

    common = dict(W=W, Wmax=Wmax, IDX_COLS=IDX_COLS, MASK_COLS=MASK_COLS,
                  NCALLS=len(per_core[0]["counts"][0]),
                  xT=xT_bf, pack0=pack0_bf, pack2=pack2_bf)
    return common, per_core


def _build_program(W, Wmax, IDX_COLS, MASK_COLS, NCALLS):
    nc = bacc.Bacc("TRN2", debug=False, num_devices=N_CORES,
                   num_swdge_queues=4,
                   dynamic_dma_scratch_size=DMA_SCRATCH)
    f32 = mybir.dt.float32
    bf16 = mybir.dt.bfloat16
    i16 = mybir.dt.int16
    i32 = mybir.dt.int32
    u16 = mybir.dt.uint16

    Wsum = W.sum(axis=1)

    xT_d = nc.dram_tensor("xT", [IN_F, TABLE_ROWS], u16,
                          kind="ExternalInput").ap()
    xperm_d = nc.dram_tensor("xperm", [IN_F, N_BLOCKS * 128], u16,
                             kind="ExternalInput").ap()
    pack0_d = nc.dram_tensor("pack0", [IN_F, HF + H], u16,
                             kind="ExternalInput").ap()
    pack2_d = nc.dram_tensor("pack2", [IN_F, HF + H], u16,
                             kind="ExternalInput").ap()
    idx_d = nc.dram_tensor("idx", [128, IDX_COLS], i16,
                           kind="ExternalInput").ap()
    cnts_d = nc.dram_tensor("cnts", [1, NCALLS], i32,
                            kind="ExternalInput").ap()
    mask_d = nc.dram_tensor("mask", [128, MASK_COLS], u16,
                            kind="ExternalInput").ap()
    out_d = nc.dram_tensor("out", [128 * N_BLOCKS, HF], f32,
                           kind="ExternalOutput").ap()
    tables = [nc.dram_tensor(f"table{c}", [CH_ROWS[c], ROW_E], bf16).ap()
              for c in range(NCH)]

    with tile.TileContext(nc) as tc, ExitStack() as ctx:
        consts = ctx.enter_context(tc.tile_pool(name="consts", bufs=1))
        bstg = ctx.enter_context(tc.tile_pool(name="bstg", bufs=2))
        rowp = ctx.enter_context(tc.tile_pool(name="rowp", bufs=2))
        idxp = ctx.enter_context(tc.tile_pool(name="idxp", bufs=3))
        xpp = ctx.enter_context(tc.tile_pool(name="xpp", bufs=2))
        gpool = ctx.enter_context(tc.tile_pool(name="gpool", bufs=G_BUFS))
        ework = ctx.enter_context(tc.tile_pool(name="ework", bufs=2))
        mwork = ctx.enter_context(tc.tile_pool(name="mwork", bufs=1))
        twork = ctx.enter_context(tc.tile_pool(name="twork", bufs=1))
        outp = ctx.enter_context(tc.tile_pool(name="outp", bufs=2))
        psA = ctx.enter_context(tc.tile_pool(name="psA", bufs=2, space="PSUM"))
        psB = ctx.enter_context(tc.tile_pool(name="psB", bufs=3, space="PSUM"))

        nc.gpsimd.load_library(mlp)

        # ---- consts ----
        pk0 = consts.tile([IN_F, HF + H], bf16)
        nc.sync.dma_start(out=pk0[:].bitcast(mybir.dt.uint16), in_=pack0_d[:])
        pk2 = consts.tile([IN_F, HF + H], bf16)
        nc.sync.dma_start(out=pk2[:].bitcast(mybir.dt.uint16), in_=pack2_d[:])
        mask_t = consts.tile([128, MASK_COLS], bf16)
        nc.sync.dma_start(out=mask_t[:].bitcast(mybir.dt.uint16), in_=mask_d[:])
        cnt_t = consts.tile([1, NCALLS], i32)
        nc.sync.dma_start(out=cnt_t[:], in_=cnts_d[:])

        # pre-zero G buffers (avoid NaN garbage in skipped gather slots)
        g_tiles = []
        for _ in range(G_BUFS):
            g = gpool.tile([128, Wmax, ROW_E], bf16, tag="G")
            nc.gpsimd.memset(g[:], 0.0)
            g_tiles.append(g)

        # ---- phase B: projection table (partition-major rows) ----
        for ch in range(NCH):
            tch = T_CH[ch]
            groups = [(g0, min(14, tch - g0)) for g0 in range(0, tch, 14)]
            for g0, gn in groups:
                xtb = bstg.tile([IN_F, 14 * 128], bf16, tag="xtb")
                c0 = CH_BASE_R[ch] + g0 * 128
                nc.sync.dma_start(
                    out=xtb[:, :gn * 128].bitcast(mybir.dt.uint16),
                    in_=xT_d[:, c0:c0 + gn * 128])
                rowb = rowp.tile([128, 14, ROW_E], bf16, tag="rowb")
                for t in range(gn):
                    pa = psA.tile([128, HF + H], f32, space="PSUM", tag="pa")
                    nc.tensor.matmul(out=pa[:],
                                     lhsT=xtb[:, t * 128:(t + 1) * 128],
                                     rhs=pk0[:], start=True, stop=True)
                    nc.scalar.activation(
                        out=rowb[:, t, 0:HF + H], in_=pa[:],
                        func=mybir.ActivationFunctionType.Copy)
                tv = tables[ch].rearrange("(p t) e -> p t e", t=tch)
                nc.sync.dma_start(out=tv[:, g0:g0 + gn, :],
                                  in_=rowb[:, :gn, :])

        # ---- phase C ----
        col_off = np.zeros(N_BLOCKS * NCH + 1, np.int64)
        np.cumsum(8 * W.reshape(-1), out=col_off[1:])
        moff_all = np.zeros(N_BLOCKS + 1, np.int64)
        np.cumsum(Wsum, out=moff_all[1:])

        regs = [nc.gpsimd.alloc_register(f"gc{i}") for i in range(4)]
        call_i = 0
        IDX_GRP = 8
        XP_GRP = 8
        OUT_GRP = 7
        outb = None
        for j in range(N_BLOCKS):
            wt = int(Wsum[j])
            if j % IDX_GRP == 0:
                g0c = int(col_off[j * NCH])
                g1c = int(col_off[min(j + IDX_GRP, N_BLOCKS) * NCH])
                idx_t = idxp.tile([128, g1c - g0c], i16, tag="idxg")
                nc.sync.dma_start(out=idx_t[:], in_=idx_d[:, g0c:g1c])
                idx_base = g0c
            if j % XP_GRP == 0:
                nxp = min(XP_GRP, N_BLOCKS - j)
                xpt = xpp.tile([IN_F, XP_GRP * 128], bf16, tag="xpt")
                nc.sync.dma_start(
                    out=xpt[:, :nxp * 128].bitcast(mybir.dt.uint16),
                    in_=xperm_d[:, j * 128:(j + nxp) * 128])
                xp_base = j
            if j % OUT_GRP == 0:
                outb = outp.tile([128, OUT_GRP, HF], f32, tag="outb")

            sk = psB.tile([128, HF + H], f32, space="PSUM", tag="sk")
            xo = (j - xp_base) * 128
            nc.tensor.matmul(out=sk[:], lhsT=xpt[:, xo:xo + 128], rhs=pk2[:],
                             start=True, stop=True)
            beta = ework.tile([128, H], bf16, tag="beta")
            nc.vector.tensor_copy(out=beta[:], in_=sk[:, HF:HF + H])

            if wt > 0:
                G = gpool.tile([128, Wmax, ROW_E], bf16, tag="G")
                cells = [ch for ch in range(NCH) if W[j, ch] > 0]
                ncell = len(cells)
                nc.gpsimd.reg_load(regs[:ncell],
                                   cnt_t[0:1, call_i:call_i + ncell])
                doff = 0
                for k, ch in enumerate(cells):
                    w = int(W[j, ch])
                    c0 = int(col_off[j * NCH + ch]) - idx_base
                    nc.gpsimd.dma_gather(
                        G[:, doff:doff + w, :], tables[ch],
                        idx_t[:, c0:c0 + 8 * w],
                        128 * w, regs[k], ROW_E,
                        single_packet=SINGLE_PACKET, queue_num=ch)
                    call_i += 1
                    doff += w

                mo = int(moff_all[j])
                DW = HF + H  # 136: [weighted p | E] fused row
                s_t = ework.tile([128, Wmax, H], f32, tag="s")
                nc.vector.tensor_tensor(
                    out=s_t[:, :wt], in0=G[:, :wt, HF:HF + H],
                    in1=beta[:].unsqueeze(1).to_broadcast([128, wt, H]),
                    op=mybir.AluOpType.add)
                e1 = ework.tile([128, Wmax, H], f32, tag="e1")
                nc.scalar.activation(out=e1[:, :wt], in_=s_t[:, :wt],
                                     func=mybir.ActivationFunctionType.Exp,
                                     scale=NEG_SLOPE)
                r_t = ework.tile([128, Wmax, H], f32, tag="rt")
                nc.scalar.activation(out=r_t[:, :wt], in_=s_t[:, :wt],
                                     func=mybir.ActivationFunctionType.Relu)
                e2 = ework.tile([128, Wmax, H], f32, tag="e2")
                nc.scalar.activation(out=e2[:, :wt], in_=r_t[:, :wt],
                                     func=mybir.ActivationFunctionType.Exp,
                                     scale=1.0 - NEG_SLOPE)
                eb = ework.tile([128, Wmax, H], bf16, tag="eb")
                nc.vector.tensor_tensor(out=eb[:, :wt], in0=e1[:, :wt],
                                        in1=e2[:, :wt],
                                        op=mybir.AluOpType.mult)
                # mp rows: [0:128] = p * E (bcast over f), [128:136] = E*mask
                mp = mwork.tile([128, Wmax, DW], bf16, tag="mp")
                nc.vector.tensor_tensor(
                    out=mp[:, :wt, HF:HF + H], in0=eb[:, :wt],
                    in1=mask_t[:, mo:mo + wt].unsqueeze(2).to_broadcast(
                        [128, wt, H]),
                    op=mybir.AluOpType.mult)
                nc.vector.tensor_tensor(
                    out=mp[:, :wt, 0:HF].rearrange("p w (h f) -> p w h f",
                                                   h=H),
                    in0=G[:, :wt, 0:HF].rearrange("p w (h f) -> p w h f",
                                                  h=H),
                    in1=mp[:, :wt, HF:HF + H].unsqueeze(3).to_broadcast(
                        [128, wt, H, F]),
                    op=mybir.AluOpType.mult)

                # bf16 tree reduce over w on [128, w, 136]; final level f32
                HT = (Wmax + 1) // 2
                tA = twork.tile([128, HT, DW], bf16, tag="tA")
                tB = twork.tile([128, HT, DW], bf16, tag="tB")
                tF = twork.tile([128, DW], f32, tag="tF")

                t = wt
                h = t // 2
                r = t - 2 * h
                if t == 1:
                    nc.vector.tensor_copy(out=tF[:], in_=mp[:, 0])
                else:
                    cur, nxt = mp, tA
                    while t > 2:
                        h = t // 2
                        r = t - 2 * h
                        nc.vector.tensor_tensor(out=nxt[:, 0:h],
                                                in0=cur[:, 0:h],
                                                in1=cur[:, h:2 * h],
                                                op=mybir.AluOpType.add)
                        if r:
                            nc.vector.tensor_copy(
                                out=nxt[:, h:h + 1],
                                in_=cur[:, 2 * h:2 * h + 1])
                        t = h + r
                        cur = nxt
                        nxt = tB if cur is tA else tA
                    nc.vector.tensor_tensor(out=tF[:], in0=cur[:, 0],
                                            in1=cur[:, 1],
                                            op=mybir.AluOpType.add)

                dinv = ework.tile([128, H], f32, tag="dinv")
                nc.vector.tensor_scalar_add(dinv[:], tF[:, HF:HF + H], EPS)
                nc.vector.reciprocal(out=dinv[:], in_=dinv[:])
                O = ework.tile([128, HF], f32, tag="O")
                nc.vector.tensor_tensor(
                    out=O[:].rearrange("p (h f) -> p h f", h=H),
                    in0=tF[:, 0:HF].rearrange("p (h f) -> p h f", h=H),
                    in1=dinv[:].unsqueeze(2).to_broadcast([128, H, F]),
                    op=mybir.AluOpType.mult)
                nc.vector.tensor_tensor(out=O[:], in0=O[:], in1=sk[:, 0:HF],
                                        op=mybir.AluOpType.add)
            else:
                O = ework.tile([128, HF], f32, tag="O")
                nc.vector.tensor_copy(out=O[:], in_=sk[:, 0:HF])

            A_t = ework.tile([128, HF], f32, tag="At")
            nc.vector.tensor_scalar(out=A_t[:], in0=O[:], scalar1=0.0,
                                    scalar2=-1.0, op0=mybir.AluOpType.max,
                                    op1=mybir.AluOpType.add)
            T_t = ework.tile([128, HF], f32, tag="Tt")
            nc.vector.tensor_scalar_min(T_t[:], O[:], 0.0)
            E2 = ework.tile([128, HF], f32, tag="E2")
            nc.scalar.activation(out=E2[:], in_=T_t[:],
                                 func=mybir.ActivationFunctionType.Exp)
            jo = j % OUT_GRP
            nc.vector.tensor_tensor(out=outb[:, jo], in0=A_t[:], in1=E2[:],
                                    op=mybir.AluOpType.add)
            if jo == OUT_GRP - 1 or j == N_BLOCKS - 1:
                j0 = j - jo
                ov = out_d.rearrange("(p j) e -> p j e", j=N_BLOCKS)
                nc.sync.dma_start(out=ov[:, j0:j + 1, :],
                                  in_=outb[:, :jo + 1, :])

    nc.compile()
    return nc


def kernel(x, edge_index, W_proj, W_skip, a_src, a_tgt):
    common, per_core = _host_prep(x, edge_index, W_proj, W_skip, a_src, a_tgt)
    key = common["W"].tobytes()
    if key not in _COMPILED:
        _COMPILED[key] = _build_program(
            common["W"], common["Wmax"], common["IDX_COLS"],
            common["MASK_COLS"], common["NCALLS"])
    nc = _COMPILED[key]

    in_maps = []
    for c in range(N_CORES):
        pc = per_core[c]
        in_maps.append({
            "xT": common["xT"],
            "xperm": pc["xperm"],
            "pack0": common["pack0"],
            "pack2": common["pack2"],
            "idx": pc["idx"],
            "cnts": pc["counts"],
            "mask": pc["mask"],
        })
    trace = bool(int(os.environ.get("GAT_TRACE", "0")))
    res = run_bass_kernel_spmd(nc, in_maps, list(range(N_CORES)),
                               trace=trace)
    if trace:
        kernel.last_exec_time_ns = res.exec_time_ns
        kernel.last_mean_exec_time_ns = res.mean_exec_time_ns

    out = np.empty((N_NODES, HF), np.float32)
    for c in range(N_CORES):
        o = res.results[c]["out"].reshape(128, N_BLOCKS, HF)
        pc = per_core[c]
        gid = pc["gid"]
        real = pc["real"]
        # slot order: k = j*128+p maps o[p, j]
        o_pj = np.transpose(o, (1, 0, 2)).reshape(N_BLOCKS * 128, HF)
        out[gid[real]] = o_pj[real]
    return out


kernel.last_exec_time_ns = None
kernel.last_mean_exec_time_ns = None


# revision 6
# speedup vs baseline: 1.2092x; 1.0134x over previous
"""GAT layer on 8 Trainium2 NeuronCores (Bass/Tile) — v2.

Target-per-partition layout (like baseline) with:
  - global count-sorted target clustering dealt round-robin to cores
    (shared static W = max over 8 cores per slot),
  - per-core runtime gather counts (trailing -1 idxs skipped by the
    SWDGE ucode) => descriptor count ~1.35x E instead of 2.06x,
  - bf16 phase B with fat (multi-KB) DMA descriptors via a
    partition-major table layout,
  - mask-based padding (no sentinel rows),
  - DVE tree-reduce instead of transposed tensor_reduce,
  - 4D inner-broadcast tensor_tensor (no Ex materialization).
"""

import os
import sys

sys.path.insert(0, "/opt/trn_rl_repo")

import numpy as np
from contextlib import ExitStack

import concourse.bass as bass
import concourse.bacc as bacc
import concourse.tile as tile
from concourse import mybir
from concourse.bass_utils import run_bass_kernel_spmd
from concourse.library_config import mlp

N_NODES = 100000
N_EDGES = 1600000
IN_F = 128
H = 8
F = 16
HF = H * F
NEG_SLOPE = 0.2
EPS = 1e-16
N_CORES = 8
N_BLOCKS = 98
N_GBLK = 784  # global blocks of 128 sorted targets
NCH = 4
CH_NODES = [25088, 25088, 25088, 24736]
T_CH = [196, 196, 196, 194]
CH_BASE_N = [0, 25088, 50176, 75264]
CH_ROWS = [128 * t for t in T_CH]  # 25088,25088,25088,24832
CH_BASE_R = [0, 25088, 50176, 75264]
TABLE_ROWS = CH_BASE_R[-1] + CH_ROWS[-1]  # 100096
ROW_E = 256  # bf16 elems per table row (512B)
XT_COLS = 100352  # 784*128, padded
ALPHA_PAD = -300.0
G_BUFS = 3
SINGLE_PACKET = False
DMA_SCRATCH = int(os.environ.get("GAT_SCRATCH", "32768"))

_COMPILED = {}


def _host_prep(x, edge_index, W_proj, W_skip, a_src, a_tgt):
    x = np.asarray(x, np.float32)
    ei = np.asarray(edge_index).astype(np.int64)
    src, tgt = ei[0], ei[1]

    ch_of = np.minimum(src // 25088, 3)
    l = src - np.take(CH_BASE_N, ch_of)
    row_of = (l % 128) * np.take(T_CH, ch_of) + l // 128  # local row in chunk

    cnt = np.zeros((N_NODES, NCH), np.int32)
    np.add.at(cnt, (tgt, ch_of), 1)
    order = np.lexsort((-cnt[:, 3], -cnt[:, 2], -cnt[:, 1], -cnt[:, 0]))
    rank = np.empty(N_NODES, np.int64)
    rank[order] = np.arange(N_NODES)
    blk = rank // 128  # global block of each target
    pos = rank % 128
    core_of = blk % 8
    slot_of = blk // 8

    cnt_sorted = np.zeros((N_GBLK * 128, NCH), np.int32)
    cnt_sorted[:N_NODES] = cnt[order]
    bm = cnt_sorted.reshape(N_GBLK, 128, NCH).max(axis=1)  # per global block
    W = bm.reshape(N_BLOCKS, 8, NCH).max(axis=1).astype(np.int32)
    Wsum = W.sum(axis=1)
    Wmax = int(Wsum.max())

    # weight packs (host-side repack of params only)
    Ablk = np.zeros((HF, H), np.float32)
    Bblk = np.zeros((HF, H), np.float32)
    asr = np.asarray(a_src, np.float32).reshape(H, F)
    atg = np.asarray(a_tgt, np.float32).reshape(H, F)
    for h in range(H):
        Ablk[h * F:(h + 1) * F, h] = asr[h]
        Bblk[h * F:(h + 1) * F, h] = atg[h]
    Wp = np.asarray(W_proj, np.float32)
    pack0 = np.concatenate([Wp, Wp @ Ablk], axis=1)  # [128,136]
    pack2 = np.concatenate([np.asarray(W_skip, np.float32), Wp @ Bblk], axis=1)

    import jax.numpy as jnp

    def to_bf16_bytes(a):
        return np.asarray(jnp.asarray(a, jnp.bfloat16)).view(np.uint16)

    xT = np.zeros((IN_F, XT_COLS), np.float32)
    xT[:, :N_NODES] = x.T
    xT_bf = to_bf16_bytes(xT[:, :TABLE_ROWS])  # device input [128, 100096] u16
    pack0_bf = to_bf16_bytes(pack0)
    pack2_bf = to_bf16_bytes(pack2)

    # per-(core, slot, chunk) edge placement
    e_core = core_of[tgt]
    e_slot = slot_of[tgt]
    e_pos = pos[tgt]
    e_ch = ch_of

    col_off = np.zeros(N_BLOCKS * NCH + 1, np.int64)  # idx col offsets (8*W)
    np.cumsum(8 * W.reshape(-1), out=col_off[1:])
    IDX_COLS = int(col_off[-1])
    MASK_COLS = int(Wsum.sum())

    per_core = []
    for c in range(N_CORES):
        m = e_core == c
        j_e = e_slot[m]
        p_e = e_pos[m]
        ch_e = e_ch[m]
        r_e = row_of[m]
        key = (j_e * 128 + p_e) * NCH + ch_e
        eo = np.argsort(key, kind="stable")
        key_s = key[eo]
        r_s = r_e[eo]
        uk, firsts = np.unique(key_s, return_index=True)
        dpos = np.arange(len(key_s)) - firsts[np.searchsorted(uk, key_s)]
        j_s = j_e[eo]
        p_s = p_e[eo]
        ch_s = ch_e[eo]

        # per-core per-block counts (for K and mask)
        cblk = np.zeros((N_BLOCKS, 128, NCH), np.int32)
        gb = 8 * np.arange(N_BLOCKS) + c
        cblk[:] = cnt_sorted.reshape(N_GBLK, 128, NCH)[gb]
        bmc = cblk.max(axis=1)  # [98, 4]
        K = np.minimum(np.maximum(bmc, (W > 0).astype(np.int32)), W)

        idxs = np.full((N_BLOCKS * NCH, ), None, dtype=object)
        counts = []
        for j in range(N_BLOCKS):
            for ch in range(NCH):
                w = int(W[j, ch])
                if w == 0:
                    continue
                k = int(K[j, ch])
                arr = np.full((w, 128), -1, np.int16)
                arr[:k, :] = 0
                idxs[j * NCH + ch] = arr
                counts.append(128 * k)
        sel = (j_s * NCH + ch_s).astype(np.int64)
        # place all edges vectorized-ish
        for cell in np.unique(sel):
            mm = sel == cell
            idxs[cell][dpos[mm], p_s[mm]] = r_s[mm]
        cols = []
        for cell in range(N_BLOCKS * NCH):
            a = idxs[cell]
            if a is None:
                continue
            flat = a.reshape(-1)
            wrap = flat.reshape(-1, 16).T
            cols.append(np.tile(wrap, (8, 1)))
        idx_mat = np.concatenate(cols, axis=1).astype(np.int16)
        assert idx_mat.shape == (128, IDX_COLS), idx_mat.shape
        counts_v = np.array(counts, np.int32)[None, :]

        # mask [128, MASK_COLS] bf16 (1.0/0.0)
        mcols = []
        for j in range(N_BLOCKS):
            for ch in range(NCH):
                w = int(W[j, ch])
                if w == 0:
                    continue
                mk = (np.arange(w)[None, :] < cblk[j, :, ch][:, None])
                mcols.append(mk)
        mask = np.concatenate(mcols, axis=1).astype(np.float32)
        mask_bf = to_bf16_bytes(mask)

        # xTperm [128, 12544] bf16: x columns of core targets in slot order
        kidx = (8 * np.arange(N_BLOCKS)[:, None] + c) * 128 + np.arange(128)[None, :]
        kidx = kidx.reshape(-1)  # [12544] global sorted rank
        gid = np.full(N_BLOCKS * 128, -1, np.int64)
        real = kidx < N_NODES
        gid[real] = order[kidx[real]]
        xp = np.zeros((IN_F, N_BLOCKS * 128), np.float32)
        xp[:, real] = x[gid[real]].T
        xperm_bf = to_bf16_bytes(xp)

        per_core.append(dict(idx=idx_mat, counts=counts_v, mask=mask_bf,
                             xperm=xperm_bf, gid=gid, real=real))

    common = dict(W=W, Wmax=Wmax, IDX_COLS=IDX_COLS, MASK_COLS=MASK_COLS,
                  NCALLS=len(per_core[0]["counts"][0]),
                  xT=xT_bf, pack0=pack0_bf, pack2=pack2_bf)
    return common, per_core


def _build_program(W, Wmax, IDX_COLS, MASK_COLS, NCALLS):
    nc = bacc.Bacc("TRN2", debug=False, num_devices=N_CORES,
                   num_swdge_queues=4,
                   dynamic_dma_scratch_size=DMA_SCRATCH)
    f32 = mybir.dt.float32
    bf16 = mybir.dt.bfloat16
    i16 = mybir.dt.int16
    i32 = mybir.dt.int32
    u16 = mybir.dt.uint16

    Wsum = W.sum(axis=1)

    xT_d = nc.dram_tensor("xT", [IN_F, TABLE_ROWS], u16,
                          kind="ExternalInput").ap()
    xperm_d = nc.dram_tensor("xperm", [IN_F, N_BLOCKS * 128], u16,
                             kind="ExternalInput").ap()
    pack0_d = nc.dram_tensor("pack0", [IN_F, HF + H], u16,
                             kind="ExternalInput").ap()
    pack2_d = nc.dram_tensor("pack2", [IN_F, HF + H], u16,
                             kind="ExternalInput").ap()
    idx_d = nc.dram_tensor("idx", [128, IDX_COLS], i16,
                           kind="ExternalInput").ap()
    cnts_d = nc.dram_tensor("cnts", [1, NCALLS], i32,
                            kind="ExternalInput").ap()
    mask_d = nc.dram_tensor("mask", [128, MASK_COLS], u16,
                            kind="ExternalInput").ap()
    out_d = nc.dram_tensor("out", [128 * N_BLOCKS, HF], f32,
                           kind="ExternalOutput").ap()
    tables = [nc.dram_tensor(f"table{c}", [CH_ROWS[c], ROW_E], bf16).ap()
              for c in range(NCH)]

    with tile.TileContext(nc) as tc, ExitStack() as ctx:
        consts = ctx.enter_context(tc.tile_pool(name="consts", bufs=1))
        bstg = ctx.enter_context(tc.tile_pool(name="bstg", bufs=2))
        rowp = ctx.enter_context(tc.tile_pool(name="rowp", bufs=2))
        idxp = ctx.enter_context(tc.tile_pool(name="idxp", bufs=3))
        xpp = ctx.enter_context(tc.tile_pool(name="xpp", bufs=2))
        gpool = ctx.enter_context(tc.tile_pool(name="gpool", bufs=G_BUFS))
        ework = ctx.enter_context(tc.tile_pool(name="ework", bufs=2))
        mwork = ctx.enter_context(tc.tile_pool(name="mwork", bufs=1))
        twork = ctx.enter_context(tc.tile_pool(name="twork", bufs=1))
        outp = ctx.enter_context(tc.tile_pool(name="outp", bufs=2))
        psA = ctx.enter_context(tc.tile_pool(name="psA", bufs=2, space="PSUM"))
        psB = ctx.enter_context(tc.tile_pool(name="psB", bufs=3, space="PSUM"))

        nc.gpsimd.load_library(mlp)

        # ---- consts ----
        pk0 = consts.tile([IN_F, HF + H], bf16)
        nc.sync.dma_start(out=pk0[:].bitcast(mybir.dt.uint16), in_=pack0_d[:])
        pk2 = consts.tile([IN_F, HF + H], bf16)
        nc.sync.dma_start(out=pk2[:].bitcast(mybir.dt.uint16), in_=pack2_d[:])
        mask_t = consts.tile([128, MASK_COLS], bf16)
        nc.sync.dma_start(out=mask_t[:].bitcast(mybir.dt.uint16), in_=mask_d[:])
        cnt_t = consts.tile([1, NCALLS], i32)
        nc.sync.dma_start(out=cnt_t[:], in_=cnts_d[:])

        # pre-zero G buffers (avoid NaN garbage in skipped gather slots)
        g_tiles = []
        for _ in range(G_BUFS):
            g = gpool.tile([128, Wmax, ROW_E], bf16, tag="G")
            nc.gpsimd.memset(g[:], 0.0)
            g_tiles.append(g)

        # ---- phase B: projection table (partition-major rows) ----
        for ch in range(NCH):
            tch = T_CH[ch]
            groups = [(g0, min(14, tch - g0)) for g0 in range(0, tch, 14)]
            for g0, gn in groups:
                xtb = bstg.tile([IN_F, 14 * 128], bf16, tag="xtb")
                c0 = CH_BASE_R[ch] + g0 * 128
                nc.sync.dma_start(
                    out=xtb[:, :gn * 128].bitcast(mybir.dt.uint16),
                    in_=xT_d[:, c0:c0 + gn * 128])
                rowb = rowp.tile([128, 14, ROW_E], bf16, tag="rowb")
                for t in range(gn):
                    pa = psA.tile([128, HF + H], f32, space="PSUM", tag="pa")
                    nc.tensor.matmul(out=pa[:],
                                     lhsT=xtb[:, t * 128:(t + 1) * 128],
                                     rhs=pk0[:], start=True, stop=True)
                    if t % 2 == 0:
                        nc.scalar.activation(
                            out=rowb[:, t, 0:HF + H], in_=pa[:],
                            func=mybir.ActivationFunctionType.Copy)
                    else:
                        nc.vector.tensor_copy(out=rowb[:, t, 0:HF + H],
                                              in_=pa[:])
                tv = tables[ch].rearrange("(p t) e -> p t e", t=tch)
                nc.sync.dma_start(out=tv[:, g0:g0 + gn, :],
                                  in_=rowb[:, :gn, :])

        # ---- phase C ----
        col_off = np.zeros(N_BLOCKS * NCH + 1, np.int64)
        np.cumsum(8 * W.reshape(-1), out=col_off[1:])
        moff_all = np.zeros(N_BLOCKS + 1, np.int64)
        np.cumsum(Wsum, out=moff_all[1:])

        regs = [nc.gpsimd.alloc_register(f"gc{i}") for i in range(4)]
        call_i = 0
        IDX_GRP = 8
        XP_GRP = 8
        OUT_GRP = 7
        outb = None
        for j in range(N_BLOCKS):
            wt = int(Wsum[j])
            if j % IDX_GRP == 0:
                g0c = int(col_off[j * NCH])
                g1c = int(col_off[min(j + IDX_GRP, N_BLOCKS) * NCH])
                idx_t = idxp.tile([128, g1c - g0c], i16, tag="idxg")
                nc.sync.dma_start(out=idx_t[:], in_=idx_d[:, g0c:g1c])
                idx_base = g0c
            if j % XP_GRP == 0:
                nxp = min(XP_GRP, N_BLOCKS - j)
                xpt = xpp.tile([IN_F, XP_GRP * 128], bf16, tag="xpt")
                nc.sync.dma_start(
                    out=xpt[:, :nxp * 128].bitcast(mybir.dt.uint16),
                    in_=xperm_d[:, j * 128:(j + nxp) * 128])
                xp_base = j
            if j % OUT_GRP == 0:
                outb = outp.tile([128, OUT_GRP, HF], f32, tag="outb")

            sk = psB.tile([128, HF + H], f32, space="PSUM", tag="sk")
            xo = (j - xp_base) * 128
            nc.tensor.matmul(out=sk[:], lhsT=xpt[:, xo:xo + 128], rhs=pk2[:],
                             start=True, stop=True)
            beta = ework.tile([128, H], bf16, tag="beta")
            nc.vector.tensor_copy(out=beta[:], in_=sk[:, HF:HF + H])

            if wt > 0:
                G = gpool.tile([128, Wmax, ROW_E], bf16, tag="G")
                cells = [ch for ch in range(NCH) if W[j, ch] > 0]
                ncell = len(cells)
                nc.gpsimd.reg_load(regs[:ncell],
                                   cnt_t[0:1, call_i:call_i + ncell])
                doff = 0
                for k, ch in enumerate(cells):
                    w = int(W[j, ch])
                    c0 = int(col_off[j * NCH + ch]) - idx_base
                    nc.gpsimd.dma_gather(
                        G[:, doff:doff + w, :], tables[ch],
                        idx_t[:, c0:c0 + 8 * w],
                        128 * w, regs[k], ROW_E,
                        single_packet=SINGLE_PACKET,
                        queue_num=(call_i % 4))
                    call_i += 1
                    doff += w

                mo = int(moff_all[j])
                DW = HF + H  # 136: [weighted p | E] fused row
                s_t = ework.tile([128, Wmax, H], f32, tag="s")
                nc.vector.tensor_tensor(
                    out=s_t[:, :wt], in0=G[:, :wt, HF:HF + H],
                    in1=beta[:].unsqueeze(1).to_broadcast([128, wt, H]),
                    op=mybir.AluOpType.add)
                e1 = ework.tile([128, Wmax, H], f32, tag="e1")
                nc.scalar.activation(out=e1[:, :wt], in_=s_t[:, :wt],
                                     func=mybir.ActivationFunctionType.Exp,
                                     scale=NEG_SLOPE)
                r_t = ework.tile([128, Wmax, H], f32, tag="rt")
                nc.scalar.activation(out=r_t[:, :wt], in_=s_t[:, :wt],
                                     func=mybir.ActivationFunctionType.Relu)
                e2 = ework.tile([128, Wmax, H], f32, tag="e2")
                nc.scalar.activation(out=e2[:, :wt], in_=r_t[:, :wt],
                                     func=mybir.ActivationFunctionType.Exp,
                                     scale=1.0 - NEG_SLOPE)
                eb = ework.tile([128, Wmax, H], bf16, tag="eb")
                nc.vector.tensor_tensor(out=eb[:, :wt], in0=e1[:, :wt],
                                        in1=e2[:, :wt],
                                        op=mybir.AluOpType.mult)
                # mp rows: [0:128] = p * E (bcast over f), [128:136] = E*mask
                mp = mwork.tile([128, Wmax, DW], bf16, tag="mp")
                nc.vector.tensor_tensor(
                    out=mp[:, :wt, HF:HF + H], in0=eb[:, :wt],
                    in1=mask_t[:, mo:mo + wt].unsqueeze(2).to_broadcast(
                        [128, wt, H]),
                    op=mybir.AluOpType.mult)
                nc.vector.tensor_tensor(
                    out=mp[:, :wt, 0:HF].rearrange("p w (h f) -> p w h f",
                                                   h=H),
                    in0=G[:, :wt, 0:HF].rearrange("p w (h f) -> p w h f",
                                                  h=H),
                    in1=mp[:, :wt, HF:HF + H].unsqueeze(3).to_broadcast(
                        [128, wt, H, F]),
                    op=mybir.AluOpType.mult)

                # bf16 tree reduce over w on [128, w, 136]; final level f32
                HT = (Wmax + 1) // 2
                tA = twork.tile([128, HT, DW], bf16, tag="tA")
                tB = twork.tile([128, HT, DW], bf16, tag="tB")
                tF = twork.tile([128, DW], f32, tag="tF")

                t = wt
                h = t // 2
                r = t - 2 * h
                if t == 1:
                    nc.vector.tensor_copy(out=tF[:], in_=mp[:, 0])
                else:
                    cur, nxt = mp, tA
                    while t > 2:
                        h = t // 2
                        r = t - 2 * h
                        nc.vector.tensor_tensor(out=nxt[:, 0:h],
                                                in0=cur[:, 0:h],
                                                in1=cur[:, h:2 * h],
                                                op=mybir.AluOpType.add)
                        if r:
                            nc.vector.tensor_copy(
                                out=nxt[:, h:h + 1],
                                in_=cur[:, 2 * h:2 * h + 1])
                        t = h + r
                        cur = nxt
                        nxt = tB if cur is tA else tA
                    nc.vector.tensor_tensor(out=tF[:], in0=cur[:, 0],
                                            in1=cur[:, 1],
                                            op=mybir.AluOpType.add)

                dinv = ework.tile([128, H], f32, tag="dinv")
                nc.vector.tensor_scalar_add(dinv[:], tF[:, HF:HF + H], EPS)
                nc.vector.reciprocal(out=dinv[:], in_=dinv[:])
                O = ework.tile([128, HF], f32, tag="O")
                nc.vector.tensor_tensor(
                    out=O[:].rearrange("p (h f) -> p h f", h=H),
                    in0=tF[:, 0:HF].rearrange("p (h f) -> p h f", h=H),
                    in1=dinv[:].unsqueeze(2).to_broadcast([128, H, F]),
                    op=mybir.AluOpType.mult)
                nc.vector.tensor_tensor(out=O[:], in0=O[:], in1=sk[:, 0:HF],
                                        op=mybir.AluOpType.add)
            else:
                O = ework.tile([128, HF], f32, tag="O")
                nc.vector.tensor_copy(out=O[:], in_=sk[:, 0:HF])

            A_t = ework.tile([128, HF], f32, tag="At")
            nc.vector.tensor_scalar(out=A_t[:], in0=O[:], scalar1=0.0,
                                    scalar2=-1.0, op0=mybir.AluOpType.max,
                                    op1=mybir.AluOpType.add)
            T_t = ework.tile([128, HF], f32, tag="Tt")
            nc.vector.tensor_scalar_min(T_t[:], O[:], 0.0)
            E2 = ework.tile([128, HF], f32, tag="E2")
            nc.scalar.activation(out=E2[:], in_=T_t[:],
                                 func=mybir.ActivationFunctionType.Exp)
            jo = j % OUT_GRP
            nc.vector.tensor_tensor(out=outb[:, jo], in0=A_t[:], in1=E2[:],
                                    op=mybir.AluOpType.add)
            if jo == OUT_GRP - 1 or j == N_BLOCKS - 1:
                j0 = j - jo
                ov = out_d.rearrange("(p j) e -> p j e", j=N_BLOCKS)
                nc.sync.dma_start(out=ov[:, j0:j + 1, :],
                                  in_=outb[:, :jo + 1, :])

    nc.compile()
    return nc


def kernel(x, edge_index, W_proj, W_skip, a_src, a_tgt):
    common, per_core = _host_prep(x, edge_index, W_proj, W_skip, a_src, a_tgt)
    key = common["W"].tobytes()
    if key not in _COMPILED:
        _COMPILED[key] = _build_program(
            common["W"], common["Wmax"], common["IDX_COLS"],
            common["MASK_COLS"], common["NCALLS"])
    nc = _COMPILED[key]

    in_maps = []
    for c in range(N_CORES):
        pc = per_core[c]
        in_maps.append({
            "xT": common["xT"],
            "xperm": pc["xperm"],
            "pack0": common["pack0"],
            "pack2": common["pack2"],
            "idx": pc["idx"],
            "cnts": pc["counts"],
            "mask": pc["mask"],
        })
    trace = bool(int(os.environ.get("GAT_TRACE", "0")))
    res = run_bass_kernel_spmd(nc, in_maps, list(range(N_CORES)),
                               trace=trace)
    if trace:
        kernel.last_exec_time_ns = res.exec_time_ns
        kernel.last_mean_exec_time_ns = res.mean_exec_time_ns

    out = np.empty((N_NODES, HF), np.float32)
    for c in range(N_CORES):
        o = res.results[c]["out"].reshape(128, N_BLOCKS, HF)
        pc = per_core[c]
        gid = pc["gid"]
        real = pc["real"]
        # slot order: k = j*128+p maps o[p, j]
        o_pj = np.transpose(o, (1, 0, 2)).reshape(N_BLOCKS * 128, HF)
        out[gid[real]] = o_pj[real]
    return out


kernel.last_exec_time_ns = None
kernel.last_mean_exec_time_ns = None
